# revision 1
# baseline (speedup 1.0000x reference)
"""AxialAttention Trainium2 kernel (8 NeuronCores, SPMD).

Sharding: core = b*4 + q  (b in {0,1}, q in {0..3}); each core handles one
batch element and a 10-row H-slab, with all 256 channels. The three
"branches" in the reference are numerically identical (h=w=d=40 and the
reshape ignores the axis names), so out = 3 * branch; the factor 3 is folded
into wp. The attention scale is folded into wq/bq.

Layouts use a "double deck": channel c (<128) lives at partition band 0-39,
channel c+128 at band 64-103 (both legal matmul base partitions). Pivot
transposes place the two decks in one psum tile via tile_position col
offsets, so psum->SBUF evacuations move both decks per op.

The slice loop is software-pipelined for the in-order PE queue:
  phase 1: pivot transposes(i) interleaved with pivot-back+wp(i-1)
  phase 2: attention(i) (scores issued one group ahead of AV) interleaved
           with the qkv conv of slice i+1.
"""

import sys

sys.path.insert(0, "/opt/trn_rl_repo")

import numpy as np
import ml_dtypes
from contextlib import ExitStack

import concourse.bass as bass
import concourse.tile as tile
from concourse import bacc, mybir
from concourse.bass_utils import run_bass_kernel_spmd
from concourse.masks import make_identity

BF16 = mybir.dt.bfloat16
F32 = mybir.dt.float32

B, C, H, W, D = 2, 256, 40, 40, 40
HEADS = 8
HD = C // HEADS
SCALE = HD ** -0.5
N_CORES = 8
SLAB = H // 4          # 10 H-rows per core
WD = W * D             # 1600
NSLAB = SLAB * WD      # 16000
CHALF = 128


def _merge(a, b):
    """Proportionally interleave two chunk lists, preserving each order."""
    out = []
    na, nb = len(a), len(b)
    ia = ib = 0
    while ia < na or ib < nb:
        if ib >= nb or (ia * (nb + 1) <= ib * (na + 1) and ia < na):
            out.append(a[ia])
            ia += 1
        else:
            out.append(b[ib])
            ib += 1
    return out


def _build_nc():
    nc = bacc.Bacc(
        "TRN2",
        target_bir_lowering=False,
        debug=False,
        num_devices=N_CORES,
    )
    x_d = nc.declare_dram_parameter("x", [C, NSLAB], BF16, isOutput=False)
    wqkv_d = nc.declare_dram_parameter("wqkv", [C, 3 * C], BF16, isOutput=False)
    bqkv_d = nc.declare_dram_parameter("bqkv", [3 * C, 1], F32, isOutput=False)
    wp_d = nc.declare_dram_parameter("wp3", [C, C], BF16, isOutput=False)
    bp_d = nc.declare_dram_parameter("bp", [C, 1], F32, isOutput=False)
    out_d = nc.declare_dram_parameter("out", [C, NSLAB], F32, isOutput=True)

    IDENT = mybir.ActivationFunctionType.Identity
    EXP = mybir.ActivationFunctionType.Exp
    MULT = mybir.AluOpType.mult

    with ExitStack() as ctx:
        tc = ctx.enter_context(tile.TileContext(nc))
        const = ctx.enter_context(tc.tile_pool(name="const", bufs=1))
        xp = ctx.enter_context(tc.tile_pool(name="xp", bufs=2))
        qkvp = ctx.enter_context(tc.tile_pool(name="qkvp", bufs=2))
        attp = ctx.enter_context(tc.tile_pool(name="attp", bufs=2))
        ep = ctx.enter_context(tc.tile_pool(name="ep", bufs=3))
        oallp = ctx.enter_context(tc.tile_pool(name="oallp", bufs=2))
        brp = ctx.enter_context(tc.tile_pool(name="brp", bufs=2))
        recp = ctx.enter_context(tc.tile_pool(name="recp", bufs=4))
        outp = ctx.enter_context(tc.tile_pool(name="outp", bufs=2))
        # conv, scores, and wp psums share one 4-deep tag (time-disjoint
        # phases); o_ps and pb share another
        ps_cs = ctx.enter_context(tc.tile_pool(name="ps_cs", bufs=4, space="PSUM"))
        ps_t = ctx.enter_context(tc.tile_pool(name="ps_t", bufs=2, space="PSUM"))
        ps_x = ctx.enter_context(tc.tile_pool(name="ps_x", bufs=2, space="PSUM"))

        ident = const.tile([128, 128], BF16)
        make_identity(nc, ident[:])

        wqkv_sb = const.tile([128, 2, 3 * C], BF16)
        nc.sync.dma_start(
            wqkv_sb[:], wqkv_d.ap().rearrange("(ko ki) m -> ki ko m", ki=128)
        )
        wp_sb = const.tile([128, 2, C], BF16)
        nc.sync.dma_start(
            wp_sb[:], wp_d.ap().rearrange("(ko ki) m -> ki ko m", ki=128)
        )
        bqkv_sb = const.tile([128, 6, 1], F32)
        nc.sync.dma_start(
            bqkv_sb[:], bqkv_d.ap().rearrange("(mo mi) one -> mi mo one", mi=128)
        )
        bp_sb = const.tile([128, 2, 1], F32)
        nc.sync.dma_start(
            bp_sb[:], bp_d.ap().rearrange("(mo mi) one -> mi mo one", mi=128)
        )

        # channel-pair groups (deck covers c_local and c_local+128)
        groups = []
        c0 = 0
        while c0 < CHALF:
            groups.append((c0, min(12, CHALF - c0)))
            c0 += 12

        def load_x(i):
            x_sb = xp.tile([128, 2, WD], BF16, name="x_sb")
            nc.sync.dma_start(
                x_sb[:],
                x_d.ap()[:, i * WD : (i + 1) * WD].rearrange(
                    "(ko ki) n -> ki ko n", ki=128
                ),
            )
            return x_sb

        def conv_emit(x_sb):
            qkv_sb = qkvp.tile([128, 6, WD], BF16, name="qkv_sb")
            chunks = []
            for m in range(6):
                for n in range(4):
                    def ch(m=m, n=n, qkv_sb=qkv_sb, x_sb=x_sb):
                        ps = ps_cs.tile(
                            [128, 512], F32, tag="ps_cs", name="conv_ps"
                        )[:, 0:400]
                        for k in range(2):
                            nc.tensor.matmul(
                                ps[:],
                                lhsT=wqkv_sb[:, k, m * 128 : (m + 1) * 128],
                                rhs=x_sb[:, k, n * 400 : (n + 1) * 400],
                                start=(k == 0),
                                stop=(k == 1),
                            )
                        if n % 2 == 0:
                            nc.vector.tensor_scalar_add(
                                qkv_sb[:, m, n * 400 : (n + 1) * 400],
                                ps[:],
                                bqkv_sb[:, m],
                            )
                        else:
                            nc.scalar.activation(
                                out=qkv_sb[:, m, n * 400 : (n + 1) * 400],
                                in_=ps[:],
                                func=IDENT,
                                bias=bqkv_sb[:, m],
                                scale=1.0,
                            )
                    chunks.append(ch)
            return qkv_sb, chunks

        def pivots_emit(qkv_sb):
            q_att = attp.tile([128, W * CHALF], BF16, tag="q_att", name="q_att")
            k_att = attp.tile([128, W * CHALF], BF16, tag="k_att", name="k_att")
            v_att = attp.tile([128, 41 * CHALF], BF16, tag="v_att", name="v_att")
            v_view = qkv_sb.rearrange("p m (w d) -> p m d w", d=40)
            chunks = [
                lambda: nc.vector.memset(v_att[:, 40 * CHALF : 41 * CHALF], 1.0)
            ]
            # q/k pivots first so the first scores' dependencies clear while
            # the v pivots (needed one stage later, at the first AV) still run
            for src, dst, eng in (
                (0, q_att, "scalar"),
                (2, k_att, "scalar"),
                (4, v_att, "vector"),
            ):
                for wg in range(10):
                    def ch(wg=wg, src=src, dst=dst, eng=eng):
                        pst = ps_t.tile([128, 512], BF16, tag="pst", name="pst")
                        for wl in range(4):
                            w = wg * 4 + wl
                            for cc in range(2):
                                r0 = cc * 64
                                if src == 4:
                                    in_ap = v_view[:, 4 + cc, w]
                                else:
                                    in_ap = qkv_sb[:, src + cc, w * 40 : (w + 1) * 40]
                                nc.tensor.transpose(
                                    pst[r0 : r0 + 40, wl * 128 : (wl + 1) * 128],
                                    in_ap,
                                    ident[:],
                                )
                        if eng == "scalar":
                            nc.scalar.copy(
                                dst[0:104, wg * 512 : (wg + 1) * 512], pst[0:104, :]
                            )
                        else:
                            nc.vector.tensor_copy(
                                out=dst[0:104, wg * 512 : (wg + 1) * 512],
                                in_=pst[0:104, :],
                            )
                    chunks.append(ch)
            return (q_att, k_att, v_att), chunks

        def attn_emit(att):
            q_att, k_att, v_att = att
            k_v = k_att.rearrange("p (w c) -> p c w", c=CHALF)
            q_v = q_att.rearrange("p (w c) -> p c w", c=CHALF)
            vv = v_att.rearrange("p (d c) -> p c d", c=CHALF)
            o_all = oallp.tile([128, CHALF * W], BF16, name="o_all")

            def scores_stage(c0, gn):
                s_ps = ps_cs.tile([128, 512], F32, tag="ps_cs", name="s_ps")
                for j in range(gn):
                    for cc in range(2):
                        r0 = cc * 64
                        nc.tensor.matmul(
                            s_ps[r0 : r0 + 40, j * 40 : (j + 1) * 40],
                            lhsT=k_v[r0 : r0 + 40, c0 + j],
                            rhs=q_v[r0 : r0 + 40, c0 + j],
                            start=True,
                            stop=True,
                        )
                e_sb = ep.tile([128, 480], BF16, tag="e_sb", name="e_sb")
                nc.scalar.activation(
                    out=e_sb[0:104, : gn * 40], in_=s_ps[0:104, : gn * 40], func=EXP
                )
                return e_sb

            def av_stage(c0, gn, e_sb):
                o_ps = ps_x.tile([128, 512], F32, tag="ps_x", name="o_ps")
                for j in range(gn):
                    for cc in range(2):
                        r0 = cc * 64
                        nc.tensor.matmul(
                            o_ps[r0 : r0 + 41, j * 40 : (j + 1) * 40],
                            lhsT=vv[r0 : r0 + 40, c0 + j],
                            rhs=e_sb[r0 : r0 + 40, j * 40 : (j + 1) * 40],
                            start=True,
                            stop=True,
                        )
                nc.vector.tensor_copy(
                    out=o_all[0:105, c0 * 40 : c0 * 40 + gn * 40],
                    in_=o_ps[0:105, : gn * 40],
                )

            pend = [None]
            chunks = []
            for c0, gn in groups:
                def ch(c0=c0, gn=gn):
                    e_sb = scores_stage(c0, gn)
                    if pend[0] is not None:
                        av_stage(*pend[0])
                    pend[0] = (c0, gn, e_sb)
                chunks.append(ch)
            chunks.append(lambda: av_stage(*pend[0]))
            return o_all, chunks

        def pbwp_emit(o_all, i):
            branch_sb = brp.tile([128, 2, WD], BF16, name="branch_sb")
            out_sb = outp.tile([128, 2, WD], F32, name="out_sb")
            o_v = o_all.rearrange("p (c w) -> p w c", w=W)
            chunks = []
            for wb in range(5):
                def ch(wb=wb):
                    pb_full = ps_x.tile([128, 1024], BF16, tag="ps_x", name="pb")
                    pb = pb_full[:, 0:848]
                    for wl in range(8):
                        w = wb * 8 + wl
                        nc.tensor.transpose(
                            pb[:, wl * 106 : wl * 106 + 105],
                            o_v[0:105, w, :],
                            ident[0:105, 0:105],
                        )
                    pb_v = pb.rearrange("p (w q) -> p w q", q=106)
                    rec = recp.tile([128, 8, 2], F32, tag="rec", name="rec")
                    nc.vector.reciprocal(rec[:, :, 0], pb_v[:, :, 40])
                    nc.vector.reciprocal(rec[:, :, 1], pb_v[:, :, 104])
                    for cc in range(2):
                        nc.vector.tensor_tensor(
                            branch_sb[:, cc].rearrange("p (w d) -> p w d", d=40)[
                                :, wb * 8 : wb * 8 + 8
                            ],
                            pb_v[:, :, cc * 64 : cc * 64 + 40],
                            rec[:, :, cc : cc + 1].to_broadcast((128, 8, 40)),
                            MULT,
                        )
                chunks.append(ch)
            for m in range(2):
                for n in range(4):
                    def ch(m=m, n=n):
                        ps = ps_cs.tile(
                            [128, 512], F32, tag="ps_cs", name="wp_ps"
                        )[:, 0:400]
                        for k in range(2):
                            nc.tensor.matmul(
                                ps[:],
                                lhsT=wp_sb[:, k, m * 128 : (m + 1) * 128],
                                rhs=branch_sb[:, k, n * 400 : (n + 1) * 400],
                                start=(k == 0),
                                stop=(k == 1),
                            )
                        if m == 0:
                            nc.scalar.activation(
                                out=out_sb[:, m, n * 400 : (n + 1) * 400],
                                in_=ps[:],
                                func=IDENT,
                                bias=bp_sb[:, m],
                                scale=1.0,
                            )
                        else:
                            nc.vector.tensor_scalar_add(
                                out_sb[:, m, n * 400 : (n + 1) * 400],
                                ps[:],
                                bp_sb[:, m],
                            )
                    chunks.append(ch)

            def dma_ch():
                nc.scalar.dma_start(
                    out_d.ap()[:, i * WD : (i + 1) * WD].rearrange(
                        "(ko ki) n -> ki ko n", ki=128
                    ),
                    out_sb[:],
                )
            chunks.append(dma_ch)
            return chunks

        # ---- software-pipelined slice loop
        x_cur = load_x(0)
        qkv_cur, conv_ch = conv_emit(x_cur)
        for ch in conv_ch:
            ch()
        pending_pbwp = []
        for i in range(SLAB):
            if i + 1 < SLAB:
                x_nxt = load_x(i + 1)
            att, piv_ch = pivots_emit(qkv_cur)
            for ch in _merge(piv_ch, pending_pbwp):
                ch()
            o_all, attn_ch = attn_emit(att)
            if i + 1 < SLAB:
                qkv_nxt, conv_ch = conv_emit(x_nxt)
            else:
                qkv_nxt, conv_ch = None, []
            for ch in _merge(attn_ch, conv_ch):
                ch()
            pending_pbwp = pbwp_emit(o_all, i)
            qkv_cur = qkv_nxt
        for ch in pending_pbwp:
            ch()

    nc.compile()
    return nc


_NC_CACHE = None


def _get_nc():
    global _NC_CACHE
    if _NC_CACHE is None:
        _NC_CACHE = _build_nc()
    return _NC_CACHE


def make_in_maps(x, wq, bq, wk, bk, wv, bv, wp, bp):
    bf = ml_dtypes.bfloat16
    wqkv = np.concatenate(
        [wq.T * SCALE, wk.T, wv.T], axis=1
    ).astype(bf)  # [C, 3C], lhsT layout (c_in rows, c_out cols)
    bqkv = np.concatenate([bq * SCALE, bk, bv]).reshape(3 * C, 1).astype(np.float32)
    wp3 = (3.0 * wp).T.astype(bf)  # [C, C]
    bp_ = bp.reshape(C, 1).astype(np.float32)
    in_maps = []
    for core in range(N_CORES):
        b = core // 4
        r0 = (core % 4) * SLAB
        x_slab = np.ascontiguousarray(
            x[b, :, r0 : r0 + SLAB].reshape(C, NSLAB)
        ).astype(bf)
        in_maps.append(
            {"x": x_slab, "wqkv": wqkv, "bqkv": bqkv, "wp3": wp3, "bp": bp_}
        )
    return in_maps


def run_on_cores(in_maps, **kw):
    nc = _get_nc()
    return run_bass_kernel_spmd(nc, in_maps, core_ids=list(range(N_CORES)), **kw)


def kernel(x, wq, bq, wk, bk, wv, bv, wp, bp):
    x = np.asarray(x, dtype=np.float32)
    in_maps = make_in_maps(
        x,
        np.asarray(wq, np.float32),
        np.asarray(bq, np.float32),
        np.asarray(wk, np.float32),
        np.asarray(bk, np.float32),
        np.asarray(wv, np.float32),
        np.asarray(bv, np.float32),
        np.asarray(wp, np.float32),
        np.asarray(bp, np.float32),
    )
    res = run_on_cores(in_maps)
    out = np.empty((B, C, H, W, D), np.float32)
    for core in range(N_CORES):
        b = core // 4
        r0 = (core % 4) * SLAB
        out[b, :, r0 : r0 + SLAB] = res.results[core]["out"].reshape(C, SLAB, W, D)
    return out


if __name__ == "__main__":
    rng = np.random.default_rng(0)
    ins = {
        "x": rng.standard_normal((B, C, H, W, D), np.float32),
        "wq": rng.standard_normal((C, C), np.float32) / 16,
        "bq": rng.standard_normal(C).astype(np.float32) * 0.01,
        "wk": rng.standard_normal((C, C), np.float32) / 16,
        "bk": rng.standard_normal(C).astype(np.float32) * 0.01,
        "wv": rng.standard_normal((C, C), np.float32) / 16,
        "bv": rng.standard_normal(C).astype(np.float32) * 0.01,
        "wp": rng.standard_normal((C, C), np.float32) / 16,
        "bp": rng.standard_normal(C).astype(np.float32) * 0.01,
    }
    out = kernel(**ins)
    print("kernel ran, out shape", out.shape, "mean", float(np.abs(out).mean()))



# revision 3
# speedup vs baseline: 1.0845x; 1.0845x over previous
"""AxialAttention Trainium2 kernel (8 NeuronCores, SPMD).

Sharding: core = b*4 + q; each core handles one batch element and a 10-row
H-slab with all 256 channels. The three reference "branches" are numerically
identical (h=w=d=40), so out = 3 * branch; the 3 is folded into wp and the
attention scale into wq/bq.

v2: all q/k/v pivot transposes run on the DMA engines' XBAR transpose
(InstDmaTransposeAnt) instead of the PE array, eliminating both the PE
transpose cost and the psum->SBUF evacuation traffic of the pivots.

H-rows are processed in PAIRS. q/k are stored [c, (w, hpair, 64-pitch-d)]
and v as [c, (d, hpair, 64-pitch-w)]; one 128-column XBAR chunk then holds
(h-even | h-odd) 64-col halves, so the transpose semantics
    out[p, 128j + c] = in[c, 128j + p]
land h-even data at partitions 0-39 and h-odd at 64-103 -- both legal
matmul partition bases. Scores/AV run per (channel, deck, h) on 40x40
tiles read straight from the transposed tiles via strided APs.
"""

import sys

sys.path.insert(0, "/opt/trn_rl_repo")

import numpy as np
import ml_dtypes
from contextlib import ExitStack

import concourse.bass as bass
import concourse.tile as tile
from concourse import bacc, mybir
from concourse.bass_utils import run_bass_kernel_spmd
from concourse.masks import make_identity

BF16 = mybir.dt.bfloat16
F32 = mybir.dt.float32

B, C, H, W, D = 2, 256, 40, 40, 40
HEADS = 8
HD = C // HEADS
SCALE = HD ** -0.5
N_CORES = 8
SLAB = H // 4           # 10 H-rows per core
WD = W * D              # 1600
NSLAB = SLAB * WD       # 16000
PAIRS = SLAB // 2       # 5 H-row pairs per core
PCOL = 2 * WD           # 3200 x/out cols per pair
PADC = W * 128          # 5120 padded cols per (tensor, deck) per pair
NSLOT = 256             # (deck, c) attention slots
GRP = 12                # slots per psum bank group
NGRP = (NSLOT + GRP - 1) // GRP


def _merge(a, b):
    """Proportionally interleave two chunk lists, preserving each order."""
    out = []
    na, nb = len(a), len(b)
    ia = ib = 0
    while ia < na or ib < nb:
        if ib >= nb or (ia * (nb + 1) <= ib * (na + 1) and ia < na):
            out.append(a[ia])
            ia += 1
        else:
            out.append(b[ib])
            ib += 1
    return out


def _build_nc():
    nc = bacc.Bacc(
        "TRN2",
        target_bir_lowering=False,
        debug=False,
        num_devices=N_CORES,
    )
    x_d = nc.declare_dram_parameter("x", [C, NSLAB], BF16, isOutput=False)
    wqkv_d = nc.declare_dram_parameter("wqkv", [C, 3 * C], BF16, isOutput=False)
    bqkv_d = nc.declare_dram_parameter("bqkv", [3 * C, 1], F32, isOutput=False)
    wp_d = nc.declare_dram_parameter("wp3", [C, C], BF16, isOutput=False)
    bp_d = nc.declare_dram_parameter("bp", [C, 1], F32, isOutput=False)
    out_d = nc.declare_dram_parameter("out", [C, NSLAB], F32, isOutput=True)

    IDENT = mybir.ActivationFunctionType.Identity
    EXP = mybir.ActivationFunctionType.Exp
    MULT = mybir.AluOpType.mult

    with ExitStack() as ctx:
        tc = ctx.enter_context(tile.TileContext(nc))
        const = ctx.enter_context(tc.tile_pool(name="const", bufs=1))
        padp = ctx.enter_context(tc.tile_pool(name="padp", bufs=1))
        ttp = ctx.enter_context(tc.tile_pool(name="ttp", bufs=8))
        ep = ctx.enter_context(tc.tile_pool(name="ep", bufs=3))
        recp = ctx.enter_context(tc.tile_pool(name="recp", bufs=4))
        ps_a = ctx.enter_context(tc.tile_pool(name="ps_a", bufs=4, space="PSUM"))
        ps_o = ctx.enter_context(tc.tile_pool(name="ps_o", bufs=2, space="PSUM"))
        ps_t = ctx.enter_context(tc.tile_pool(name="ps_t", bufs=2, space="PSUM"))

        ident = const.tile([128, 128], BF16)
        make_identity(nc, ident[:])

        wqkv_sb = const.tile([128, 2, 3 * C], BF16)
        nc.sync.dma_start(
            wqkv_sb[:], wqkv_d.ap().rearrange("(ko ki) m -> ki ko m", ki=128)
        )
        wp_sb = const.tile([128, 2, C], BF16)
        nc.sync.dma_start(
            wp_sb[:], wp_d.ap().rearrange("(ko ki) m -> ki ko m", ki=128)
        )
        bqkv_sb = const.tile([128, 6, 1], F32)
        nc.sync.dma_start(
            bqkv_sb[:], bqkv_d.ap().rearrange("(mo mi) one -> mi mo one", mi=128)
        )
        bp_sb = const.tile([128, 2, 1], F32)
        nc.sync.dma_start(
            bp_sb[:], bp_d.ap().rearrange("(mo mi) one -> mi mo one", mi=128)
        )

        # single-buffered pair-state
        x_sb = padp.tile([128, 2, PCOL], BF16)
        q_pad = padp.tile([128, 2, PADC], BF16)
        k_pad = padp.tile([128, 2, PADC], BF16)
        v_pad = padp.tile([128, 2, PADC], BF16)
        o_all = padp.tile([128, NSLOT * W], BF16)
        branch = padp.tile([128, 2, PCOL], BF16)
        out_q = padp.tile([128, 2, 2 * 400], F32)

        def load_x(i):
            nc.sync.dma_start(
                x_sb[:],
                x_d.ap()[:, i * PCOL : (i + 1) * PCOL].rearrange(
                    "(ko ki) n -> ki ko n", ki=128
                ),
            )

        # transposed-tile slots: [128, 41, 128]; chunk 40 holds the persistent
        # ones column used by AV to produce softmax denominators.
        def tslot():
            t = ttp.tile([128, 41, 128], BF16, tag="tt", name="tslot")
            return t

        def conv_emit():
            """qkv conv of the pair in x_sb into padded q/k/v + transpose DMAs.
            Returns chunk list; after it runs, q/k/v transposed slots are being
            DMA-filled (slots returned immediately)."""
            slots = {}
            for t in range(3):
                for deck in range(2):
                    slots[(t, deck)] = None
            chunks = []
            dst_pad = (q_pad, k_pad, v_pad)

            for m in range(6):
                tn, deck = m // 2, m % 2
                for n in range(8):
                    def ch(tn=tn, deck=deck, n=n, m=m):
                        ps = ps_a.tile(
                            [128, 512], F32, tag="ps_a", name="conv_ps"
                        )[:, 0:400]
                        for k in range(2):
                            nc.tensor.matmul(
                                ps[:],
                                lhsT=wqkv_sb[:, k, m * 128 : (m + 1) * 128],
                                rhs=x_sb[:, k, n * 400 : (n + 1) * 400],
                                start=(k == 0),
                                stop=(k == 1),
                            )
                        h, w0 = n // 4, 10 * (n % 4)
                        pad = dst_pad[tn]
                        if tn < 2:
                            # q/k: col = w*128 + h*64 + d
                            dst = pad.rearrange(
                                "p k (w x) -> p k w x", x=128
                            )[:, deck, w0 : w0 + 10, 64 * h : 64 * h + 40]
                        else:
                            # v: col = d*128 + h*64 + w
                            dst = pad.rearrange(
                                "p k (d x) -> p k x d", x=128
                            )[:, deck, 64 * h + w0 : 64 * h + w0 + 10, :]
                        ps_v = ps.rearrange("p (w d) -> p w d", d=40)
                        if n % 2 == 0:
                            nc.vector.tensor_scalar_add(dst, ps_v, bqkv_sb[:, m])
                        else:
                            nc.scalar.activation(
                                out=dst, in_=ps_v, func=IDENT,
                                bias=bqkv_sb[:, m], scale=1.0,
                            )
                    chunks.append(ch)

                def dma_ch(tn=tn, deck=deck, slots=slots):
                    t = tslot()
                    slots[(tn, deck)] = t
                    nc.sync.dma_start(
                        t[:, 0:40, :], dst_pad[tn][:, deck, :], transpose=True
                    )
                chunks.append(dma_ch)
            return slots, chunks

        def attn_emit(slots):
            """scores -> exp -> AV per 12-slot group, o_all evacuation."""
            qs = (slots[(0, 0)], slots[(0, 1)])
            ks = (slots[(1, 0)], slots[(1, 1)])
            vs = (slots[(2, 0)], slots[(2, 1)])

            def scores_stage(g):
                s0 = g * GRP
                gn = min(GRP, NSLOT - s0)
                s_ps = ps_a.tile([128, 512], F32, tag="ps_a", name="s_ps")
                for j in range(gn):
                    s = s0 + j
                    deck, c = s // 128, s % 128
                    for hp in range(2):
                        r = 64 * hp
                        nc.tensor.matmul(
                            s_ps[r : r + 40, j * 40 : (j + 1) * 40],
                            lhsT=ks[deck][r : r + 40, 0:40, c],
                            rhs=qs[deck][r : r + 40, 0:40, c],
                            start=True,
                            stop=True,
                        )
                e_sb = ep.tile([128, 480], BF16, tag="e_sb", name="e_sb")
                nc.scalar.activation(
                    out=e_sb[0:104, : gn * 40],
                    in_=s_ps[0:104, : gn * 40],
                    func=EXP,
                )
                return e_sb

            def av_stage(g, e_sb):
                s0 = g * GRP
                gn = min(GRP, NSLOT - s0)
                o_ps = ps_o.tile([128, 512], F32, tag="ps_o", name="o_ps")
                for j in range(gn):
                    s = s0 + j
                    deck, c = s // 128, s % 128
                    for hp in range(2):
                        r = 64 * hp
                        nc.tensor.matmul(
                            o_ps[r : r + 41, j * 40 : (j + 1) * 40],
                            lhsT=vs[deck][r : r + 40, 0:41, c],
                            rhs=e_sb[r : r + 40, j * 40 : (j + 1) * 40],
                            start=True,
                            stop=True,
                        )
                nc.vector.tensor_copy(
                    out=o_all[0:105, s0 * 40 : (s0 + gn) * 40],
                    in_=o_ps[0:105, : gn * 40],
                )

            pend = [None]
            chunks = []
            for g in range(NGRP):
                def ch(g=g):
                    e_sb = scores_stage(g)
                    if pend[0] is not None:
                        av_stage(*pend[0])
                    pend[0] = (g, e_sb)
                chunks.append(ch)
            chunks.append(lambda: av_stage(*pend[0]))
            return chunks

        def pbwp_emit(i):
            """pivot-back + normalize + wp conv + out DMA for pair i."""
            o_v = o_all.rearrange("p (s w) -> p w s", w=W)
            chunks = []
            # 10 pivot-back chunks: (deck, 8-w group)
            for pc in range(10):
                def ch(pc=pc):
                    deck, w0 = pc // 5, 8 * (pc % 5)
                    pb_full = ps_t.tile([128, 1024], BF16, tag="ps_t", name="pb")
                    pb = pb_full[:, 0:848]
                    for wl in range(8):
                        w = w0 + wl
                        nc.tensor.transpose(
                            pb[:, wl * 106 : wl * 106 + 105],
                            o_v[0:105, w, deck * 128 : deck * 128 + 128],
                            ident[0:105, 0:105],
                        )
                    pb_v = pb.rearrange("p (w q) -> p w q", q=106)
                    rec = recp.tile([128, 8, 2], F32, tag="rec", name="rec")
                    nc.vector.reciprocal(rec[:, :, 0], pb_v[:, :, 40])
                    nc.vector.reciprocal(rec[:, :, 1], pb_v[:, :, 104])
                    for hp in range(2):
                        nc.vector.tensor_tensor(
                            branch[:, deck, hp * WD : (hp + 1) * WD].rearrange(
                                "p (w d) -> p w d", d=40
                            )[:, w0 : w0 + 8],
                            pb_v[:, :, 64 * hp : 64 * hp + 40],
                            rec[:, :, hp : hp + 1].to_broadcast((128, 8, 40)),
                            MULT,
                        )
                chunks.append(ch)
            # wp: quarter-pair granularity (2 n-chunks x 2 m), then DMA out
            for q in range(4):
                for nl in range(2):
                    for m in range(2):
                        def ch(q=q, nl=nl, m=m):
                            n = 2 * q + nl
                            ps = ps_a.tile(
                                [128, 512], F32, tag="ps_a", name="wp_ps"
                            )[:, 0:400]
                            for k in range(2):
                                nc.tensor.matmul(
                                    ps[:],
                                    lhsT=wp_sb[:, k, m * 128 : (m + 1) * 128],
                                    rhs=branch[:, k, n * 400 : (n + 1) * 400],
                                    start=(k == 0),
                                    stop=(k == 1),
                                )
                            if (n + m) % 2 == 0:
                                nc.scalar.activation(
                                    out=out_q[:, m, nl * 400 : (nl + 1) * 400],
                                    in_=ps[:], func=IDENT,
                                    bias=bp_sb[:, m], scale=1.0,
                                )
                            else:
                                nc.vector.tensor_scalar_add(
                                    out_q[:, m, nl * 400 : (nl + 1) * 400],
                                    ps[:], bp_sb[:, m],
                                )
                        chunks.append(ch)

                def dma_ch(q=q, i=i):
                    nc.sync.dma_start(
                        out_d.ap()[
                            :, i * PCOL + q * 800 : i * PCOL + (q + 1) * 800
                        ].rearrange("(ko ki) n -> ki ko n", ki=128),
                        out_q[:],
                    )
                chunks.append(dma_ch)
            return chunks

        # ones columns for AV softmax denominators: chunk 40 of every slot.
        # Slots are written by transpose DMAs only in chunks 0-39, so these
        # persist across the whole kernel.
        first = [ttp.tile([128, 41, 128], BF16, tag="tt", name="tslot")
                 for _ in range(8)]
        for t in first:
            nc.vector.memset(t[:, 40, :], 1.0)
        del first  # pool rotation reuses these 8 slots

        # ---- software-pipelined pair loop
        load_x(0)
        slots_cur, conv_ch = conv_emit()
        for ch in conv_ch:
            ch()
        pending_pbwp = []
        for i in range(PAIRS):
            attn_ch = attn_emit(slots_cur)
            if i + 1 < PAIRS:
                def ch_load(i=i):
                    load_x(i + 1)
                slots_nxt, conv_ch = conv_emit()
                other = _merge(pending_pbwp, [ch_load] + conv_ch)
            else:
                slots_nxt, other = None, pending_pbwp
            for ch in _merge(attn_ch, other):
                ch()
            pending_pbwp = pbwp_emit(i)
            slots_cur = slots_nxt
        for ch in pending_pbwp:
            ch()

    nc.compile()
    return nc


_NC_CACHE = None


def _get_nc():
    global _NC_CACHE
    if _NC_CACHE is None:
        _NC_CACHE = _build_nc()
    return _NC_CACHE


def make_in_maps(x, wq, bq, wk, bk, wv, bv, wp, bp):
    bf = ml_dtypes.bfloat16
    wqkv = np.concatenate(
        [wq.T * SCALE, wk.T, wv.T], axis=1
    ).astype(bf)  # [C, 3C], lhsT layout (c_in rows, c_out cols)
    bqkv = np.concatenate([bq * SCALE, bk, bv]).reshape(3 * C, 1).astype(np.float32)
    wp3 = (3.0 * wp).T.astype(bf)  # [C, C]
    bp_ = bp.reshape(C, 1).astype(np.float32)
    in_maps = []
    for core in range(N_CORES):
        b = core // 4
        r0 = (core % 4) * SLAB
        x_slab = np.ascontiguousarray(
            x[b, :, r0 : r0 + SLAB].reshape(C, NSLAB)
        ).astype(bf)
        in_maps.append(
            {"x": x_slab, "wqkv": wqkv, "bqkv": bqkv, "wp3": wp3, "bp": bp_}
        )
    return in_maps


def run_on_cores(in_maps, **kw):
    nc = _get_nc()
    return run_bass_kernel_spmd(nc, in_maps, core_ids=list(range(N_CORES)), **kw)


def kernel(x, wq, bq, wk, bk, wv, bv, wp, bp):
    x = np.asarray(x, dtype=np.float32)
    in_maps = make_in_maps(
        x,
        np.asarray(wq, np.float32),
        np.asarray(bq, np.float32),
        np.asarray(wk, np.float32),
        np.asarray(bk, np.float32),
        np.asarray(wv, np.float32),
        np.asarray(bv, np.float32),
        np.asarray(wp, np.float32),
        np.asarray(bp, np.float32),
    )
    res = run_on_cores(in_maps)
    out = np.empty((B, C, H, W, D), np.float32)
    for core in range(N_CORES):
        b = core // 4
        r0 = (core % 4) * SLAB
        out[b, :, r0 : r0 + SLAB] = res.results[core]["out"].reshape(C, SLAB, W, D)
    return out


if __name__ == "__main__":
    rng = np.random.default_rng(0)
    ins = {
        "x": rng.standard_normal((B, C, H, W, D), np.float32),
        "wq": rng.standard_normal((C, C), np.float32) / 16,
        "bq": rng.standard_normal(C).astype(np.float32) * 0.01,
        "wk": rng.standard_normal((C, C), np.float32) / 16,
        "bk": rng.standard_normal(C).astype(np.float32) * 0.01,
        "wv": rng.standard_normal((C, C), np.float32) / 16,
        "bv": rng.standard_normal(C).astype(np.float32) * 0.01,
        "wp": rng.standard_normal((C, C), np.float32) / 16,
        "bp": rng.standard_normal(C).astype(np.float32) * 0.01,
    }
    out = kernel(**ins)
    print("kernel ran, out shape", out.shape, "mean", float(np.abs(out).mean()))


# revision 4
# speedup vs baseline: 1.1083x; 1.0219x over previous
"""AxialAttention Trainium2 kernel (8 NeuronCores, SPMD).

Sharding: core = b*4 + q; each core handles one batch element and a 10-row
H-slab with all 256 channels. The three reference "branches" are numerically
identical (h=w=d=40), so out = 3 * branch; the 3 is folded into wp and the
attention scale into wq/bq.

v2: all q/k/v pivot transposes run on the DMA engines' XBAR transpose
(InstDmaTransposeAnt) instead of the PE array, eliminating both the PE
transpose cost and the psum->SBUF evacuation traffic of the pivots.

H-rows are processed in PAIRS. q/k are stored [c, (w, hpair, 64-pitch-d)]
and v as [c, (d, hpair, 64-pitch-w)]; one 128-column XBAR chunk then holds
(h-even | h-odd) 64-col halves, so the transpose semantics
    out[p, 128j + c] = in[c, 128j + p]
land h-even data at partitions 0-39 and h-odd at 64-103 -- both legal
matmul partition bases. Scores/AV run per (channel, deck, h) on 40x40
tiles read straight from the transposed tiles via strided APs.
"""

import sys

sys.path.insert(0, "/opt/trn_rl_repo")

import numpy as np
import ml_dtypes
from contextlib import ExitStack

import concourse.bass as bass
import concourse.tile as tile
from concourse import bacc, mybir
from concourse.bass_utils import run_bass_kernel_spmd
from concourse.masks import make_identity

BF16 = mybir.dt.bfloat16
F32 = mybir.dt.float32

B, C, H, W, D = 2, 256, 40, 40, 40
HEADS = 8
HD = C // HEADS
SCALE = HD ** -0.5
N_CORES = 8
SLAB = H // 4           # 10 H-rows per core
WD = W * D              # 1600
NSLAB = SLAB * WD       # 16000
PAIRS = SLAB // 2       # 5 H-row pairs per core
PCOL = 2 * WD           # 3200 x/out cols per pair
PADC = W * 128          # 5120 padded cols per (tensor, deck) per pair
NSLOT = 256             # (deck, c) attention slots
GRP = 12                # slots per psum bank group
NGRP = (NSLOT + GRP - 1) // GRP


def _merge(a, b):
    """Proportionally interleave two chunk lists, preserving each order."""
    out = []
    na, nb = len(a), len(b)
    ia = ib = 0
    while ia < na or ib < nb:
        if ib >= nb or (ia * (nb + 1) <= ib * (na + 1) and ia < na):
            out.append(a[ia])
            ia += 1
        else:
            out.append(b[ib])
            ib += 1
    return out


def _build_nc():
    nc = bacc.Bacc(
        "TRN2",
        target_bir_lowering=False,
        debug=False,
        num_devices=N_CORES,
    )
    x_d = nc.declare_dram_parameter("x", [C, NSLAB], BF16, isOutput=False)
    wqkv_d = nc.declare_dram_parameter("wqkv", [C, 3 * C], BF16, isOutput=False)
    bqkv_d = nc.declare_dram_parameter("bqkv", [3 * C, 1], F32, isOutput=False)
    wp_d = nc.declare_dram_parameter("wp3", [C, C], BF16, isOutput=False)
    bp_d = nc.declare_dram_parameter("bp", [C, 1], F32, isOutput=False)
    out_d = nc.declare_dram_parameter("out", [C, NSLAB], F32, isOutput=True)

    IDENT = mybir.ActivationFunctionType.Identity
    EXP = mybir.ActivationFunctionType.Exp
    MULT = mybir.AluOpType.mult

    with ExitStack() as ctx:
        tc = ctx.enter_context(tile.TileContext(nc))
        const = ctx.enter_context(tc.tile_pool(name="const", bufs=1))
        padp = ctx.enter_context(tc.tile_pool(name="padp", bufs=1))
        ttp = ctx.enter_context(tc.tile_pool(name="ttp", bufs=8))
        ep = ctx.enter_context(tc.tile_pool(name="ep", bufs=3))
        recp = ctx.enter_context(tc.tile_pool(name="recp", bufs=4))
        ps_a = ctx.enter_context(tc.tile_pool(name="ps_a", bufs=4, space="PSUM"))
        ps_o = ctx.enter_context(tc.tile_pool(name="ps_o", bufs=2, space="PSUM"))
        ps_t = ctx.enter_context(tc.tile_pool(name="ps_t", bufs=2, space="PSUM"))

        ident = const.tile([128, 128], BF16)
        make_identity(nc, ident[:])

        wqkv_sb = const.tile([128, 2, 3 * C], BF16)
        nc.sync.dma_start(
            wqkv_sb[:], wqkv_d.ap().rearrange("(ko ki) m -> ki ko m", ki=128)
        )
        wp_sb = const.tile([128, 2, C], BF16)
        nc.sync.dma_start(
            wp_sb[:], wp_d.ap().rearrange("(ko ki) m -> ki ko m", ki=128)
        )
        bqkv_sb = const.tile([128, 6, 1], F32)
        nc.sync.dma_start(
            bqkv_sb[:], bqkv_d.ap().rearrange("(mo mi) one -> mi mo one", mi=128)
        )
        bp_sb = const.tile([128, 2, 1], F32)
        nc.sync.dma_start(
            bp_sb[:], bp_d.ap().rearrange("(mo mi) one -> mi mo one", mi=128)
        )

        # single-buffered pair-state
        x_sb = padp.tile([128, 2, PCOL], BF16)
        q_pad = padp.tile([128, 2, PADC], BF16)
        k_pad = padp.tile([128, 2, PADC], BF16)
        v_pad = padp.tile([128, 2, PADC], BF16)
        o_all = padp.tile([128, NSLOT * W], BF16)
        branch = padp.tile([128, 2, PCOL], BF16)
        out_q = padp.tile([128, 2, 2 * 400], F32)

        def load_x(i):
            nc.sync.dma_start(
                x_sb[:],
                x_d.ap()[:, i * PCOL : (i + 1) * PCOL].rearrange(
                    "(ko ki) n -> ki ko n", ki=128
                ),
            )

        # transposed-tile slots: [128, 41, 128]; chunk 40 holds the persistent
        # ones column used by AV to produce softmax denominators.
        def tslot():
            t = ttp.tile([128, 41, 128], BF16, tag="tt", name="tslot")
            return t

        def conv_emit():
            """qkv conv of the pair in x_sb into padded q/k/v + transpose DMAs.
            Returns chunk list; after it runs, q/k/v transposed slots are being
            DMA-filled (slots returned immediately)."""
            slots = {}
            for t in range(3):
                for deck in range(2):
                    slots[(t, deck)] = None
            chunks = []
            dst_pad = (q_pad, k_pad, v_pad)

            for m in range(6):
                tn, deck = m // 2, m % 2
                for n in range(8):
                    def ch(tn=tn, deck=deck, n=n, m=m):
                        ps = ps_a.tile(
                            [128, 512], F32, tag="ps_a", name="conv_ps"
                        )[:, 0:400]
                        for k in range(2):
                            nc.tensor.matmul(
                                ps[:],
                                lhsT=wqkv_sb[:, k, m * 128 : (m + 1) * 128],
                                rhs=x_sb[:, k, n * 400 : (n + 1) * 400],
                                start=(k == 0),
                                stop=(k == 1),
                            )
                        h, w0 = n // 4, 10 * (n % 4)
                        pad = dst_pad[tn]
                        if tn < 2:
                            # q/k: col = w*128 + h*64 + d
                            dst = pad.rearrange(
                                "p k (w x) -> p k w x", x=128
                            )[:, deck, w0 : w0 + 10, 64 * h : 64 * h + 40]
                        else:
                            # v: col = d*128 + h*64 + w
                            dst = pad.rearrange(
                                "p k (d x) -> p k x d", x=128
                            )[:, deck, 64 * h + w0 : 64 * h + w0 + 10, :]
                        ps_v = ps.rearrange("p (w d) -> p w d", d=40)
                        if n % 2 == 0:
                            nc.vector.tensor_scalar_add(dst, ps_v, bqkv_sb[:, m])
                        else:
                            nc.scalar.activation(
                                out=dst, in_=ps_v, func=IDENT,
                                bias=bqkv_sb[:, m], scale=1.0,
                            )
                    chunks.append(ch)

                def dma_ch(tn=tn, deck=deck, slots=slots):
                    t = tslot()
                    slots[(tn, deck)] = t
                    nc.sync.dma_start(
                        t[:, 0:40, :], dst_pad[tn][:, deck, :], transpose=True
                    )
                chunks.append(dma_ch)
            return slots, chunks

        def attn_emit(slots):
            """scores -> exp -> AV per 12-slot group, o_all evacuation."""
            qs = (slots[(0, 0)], slots[(0, 1)])
            ks = (slots[(1, 0)], slots[(1, 1)])
            vs = (slots[(2, 0)], slots[(2, 1)])

            def scores_stage(g):
                s0 = g * GRP
                gn = min(GRP, NSLOT - s0)
                s_ps = ps_a.tile([128, 512], F32, tag="ps_a", name="s_ps")
                for j in range(gn):
                    s = s0 + j
                    deck, c = s // 128, s % 128
                    for hp in range(2):
                        r = 64 * hp
                        nc.tensor.matmul(
                            s_ps[r : r + 40, j * 40 : (j + 1) * 40],
                            lhsT=ks[deck][r : r + 40, 0:40, c],
                            rhs=qs[deck][r : r + 40, 0:40, c],
                            start=True,
                            stop=True,
                        )
                e_sb = ep.tile([128, 480], BF16, tag="e_sb", name="e_sb")
                nc.scalar.activation(
                    out=e_sb[0:104, : gn * 40],
                    in_=s_ps[0:104, : gn * 40],
                    func=EXP,
                )
                return e_sb

            def av_stage(g, e_sb):
                s0 = g * GRP
                gn = min(GRP, NSLOT - s0)
                o_ps = ps_o.tile([128, 512], F32, tag="ps_o", name="o_ps")
                for j in range(gn):
                    s = s0 + j
                    deck, c = s // 128, s % 128
                    for hp in range(2):
                        r = 64 * hp
                        nc.tensor.matmul(
                            o_ps[r : r + 41, j * 40 : (j + 1) * 40],
                            lhsT=vs[deck][r : r + 40, 0:41, c],
                            rhs=e_sb[r : r + 40, j * 40 : (j + 1) * 40],
                            start=True,
                            stop=True,
                        )
                nc.vector.tensor_copy(
                    out=o_all[0:105, s0 * 40 : (s0 + gn) * 40],
                    in_=o_ps[0:105, : gn * 40],
                )

            pend = [None]
            chunks = []
            for g in range(NGRP):
                def ch(g=g):
                    e_sb = scores_stage(g)
                    if pend[0] is not None:
                        av_stage(*pend[0])
                    pend[0] = (g, e_sb)
                chunks.append(ch)
            chunks.append(lambda: av_stage(*pend[0]))
            return chunks

        def pbwp_emit(i):
            """pivot-back + normalize + wp conv + out DMA for pair i."""
            o_v = o_all.rearrange("p (s w) -> p w s", w=W)
            chunks = []
            # 10 pivot-back chunks: (deck, 8-w group)
            for pc in range(10):
                def ch(pc=pc):
                    deck, w0 = pc // 5, 8 * (pc % 5)
                    pb_full = ps_t.tile([128, 1024], BF16, tag="ps_t", name="pb")
                    pb = pb_full[:, 0:848]
                    for wl in range(8):
                        w = w0 + wl
                        nc.tensor.transpose(
                            pb[:, wl * 106 : wl * 106 + 105],
                            o_v[0:105, w, deck * 128 : deck * 128 + 128],
                            ident[0:105, 0:105],
                        )
                    pb_v = pb.rearrange("p (w q) -> p w q", q=106)
                    rec = recp.tile([128, 8, 2], F32, tag="rec", name="rec")
                    nc.vector.reciprocal(rec[:, :, 0], pb_v[:, :, 40])
                    nc.vector.reciprocal(rec[:, :, 1], pb_v[:, :, 104])
                    for hp in range(2):
                        nc.vector.tensor_tensor(
                            branch[:, deck, hp * WD : (hp + 1) * WD].rearrange(
                                "p (w d) -> p w d", d=40
                            )[:, w0 : w0 + 8],
                            pb_v[:, :, 64 * hp : 64 * hp + 40],
                            rec[:, :, hp : hp + 1].to_broadcast((128, 8, 40)),
                            MULT,
                        )
                chunks.append(ch)
            # wp: quarter-pair granularity (2 n-chunks x 2 m), then DMA out
            for q in range(4):
                for nl in range(2):
                    for m in range(2):
                        def ch(q=q, nl=nl, m=m):
                            n = 2 * q + nl
                            ps = ps_a.tile(
                                [128, 512], F32, tag="ps_a", name="wp_ps"
                            )[:, 0:400]
                            for k in range(2):
                                nc.tensor.matmul(
                                    ps[:],
                                    lhsT=wp_sb[:, k, m * 128 : (m + 1) * 128],
                                    rhs=branch[:, k, n * 400 : (n + 1) * 400],
                                    start=(k == 0),
                                    stop=(k == 1),
                                )
                            if (n + m) % 2 == 0:
                                nc.scalar.activation(
                                    out=out_q[:, m, nl * 400 : (nl + 1) * 400],
                                    in_=ps[:], func=IDENT,
                                    bias=bp_sb[:, m], scale=1.0,
                                )
                            else:
                                nc.vector.tensor_scalar_add(
                                    out_q[:, m, nl * 400 : (nl + 1) * 400],
                                    ps[:], bp_sb[:, m],
                                )
                        chunks.append(ch)

                def dma_ch(q=q, i=i):
                    nc.sync.dma_start(
                        out_d.ap()[
                            :, i * PCOL + q * 800 : i * PCOL + (q + 1) * 800
                        ].rearrange("(ko ki) n -> ki ko n", ki=128),
                        out_q[:],
                    )
                chunks.append(dma_ch)
            return chunks

        # ones columns for AV softmax denominators: chunk 40 of every slot.
        # Slots are written by transpose DMAs only in chunks 0-39, so these
        # persist across the whole kernel.
        first = [ttp.tile([128, 41, 128], BF16, tag="tt", name="tslot")
                 for _ in range(8)]
        for t in first:
            nc.vector.memset(t[:, 40, :], 1.0)
        del first  # pool rotation reuses these 8 slots

        # ---- software-pipelined pair loop
        # o_all/branch/x are single-buffered, so ordering is load-bearing:
        # all pbwp(i-1) chunks (which read o_all of pair i-1) must be emitted
        # before attn(i)'s o_all evacuations (engines execute in order).
        load_x(0)
        slots_cur, conv_ch = conv_emit()
        for ch in conv_ch:
            ch()
        pending_pbwp = []
        for i in range(PAIRS):
            if i + 1 < PAIRS:
                def ch_load(i=i):
                    load_x(i + 1)
                slots_nxt, conv_ch = conv_emit()
                pre = _merge(pending_pbwp, [ch_load] + conv_ch)
            else:
                slots_nxt, pre = None, pending_pbwp
            for ch in pre:
                ch()
            for ch in attn_emit(slots_cur):
                ch()
            pending_pbwp = pbwp_emit(i)
            slots_cur = slots_nxt
        for ch in pending_pbwp:
            ch()

    nc.compile()
    return nc


_NC_CACHE = None


def _get_nc():
    global _NC_CACHE
    if _NC_CACHE is None:
        _NC_CACHE = _build_nc()
    return _NC_CACHE


def make_in_maps(x, wq, bq, wk, bk, wv, bv, wp, bp):
    bf = ml_dtypes.bfloat16
    wqkv = np.concatenate(
        [wq.T * SCALE, wk.T, wv.T], axis=1
    ).astype(bf)  # [C, 3C], lhsT layout (c_in rows, c_out cols)
    bqkv = np.concatenate([bq * SCALE, bk, bv]).reshape(3 * C, 1).astype(np.float32)
    wp3 = (3.0 * wp).T.astype(bf)  # [C, C]
    bp_ = bp.reshape(C, 1).astype(np.float32)
    in_maps = []
    for core in range(N_CORES):
        b = core // 4
        r0 = (core % 4) * SLAB
        x_slab = np.ascontiguousarray(
            x[b, :, r0 : r0 + SLAB].reshape(C, NSLAB)
        ).astype(bf)
        in_maps.append(
            {"x": x_slab, "wqkv": wqkv, "bqkv": bqkv, "wp3": wp3, "bp": bp_}
        )
    return in_maps


def run_on_cores(in_maps, **kw):
    nc = _get_nc()
    return run_bass_kernel_spmd(nc, in_maps, core_ids=list(range(N_CORES)), **kw)


def kernel(x, wq, bq, wk, bk, wv, bv, wp, bp):
    x = np.asarray(x, dtype=np.float32)
    in_maps = make_in_maps(
        x,
        np.asarray(wq, np.float32),
        np.asarray(bq, np.float32),
        np.asarray(wk, np.float32),
        np.asarray(bk, np.float32),
        np.asarray(wv, np.float32),
        np.asarray(bv, np.float32),
        np.asarray(wp, np.float32),
        np.asarray(bp, np.float32),
    )
    res = run_on_cores(in_maps)
    out = np.empty((B, C, H, W, D), np.float32)
    for core in range(N_CORES):
        b = core // 4
        r0 = (core % 4) * SLAB
        out[b, :, r0 : r0 + SLAB] = res.results[core]["out"].reshape(C, SLAB, W, D)
    return out


if __name__ == "__main__":
    rng = np.random.default_rng(0)
    ins = {
        "x": rng.standard_normal((B, C, H, W, D), np.float32),
        "wq": rng.standard_normal((C, C), np.float32) / 16,
        "bq": rng.standard_normal(C).astype(np.float32) * 0.01,
        "wk": rng.standard_normal((C, C), np.float32) / 16,
        "bk": rng.standard_normal(C).astype(np.float32) * 0.01,
        "wv": rng.standard_normal((C, C), np.float32) / 16,
        "bv": rng.standard_normal(C).astype(np.float32) * 0.01,
        "wp": rng.standard_normal((C, C), np.float32) / 16,
        "bp": rng.standard_normal(C).astype(np.float32) * 0.01,
    }
    out = kernel(**ins)
    print("kernel ran, out shape", out.shape, "mean", float(np.abs(out).mean()))


# revision 12
# speedup vs baseline: 1.1490x; 1.0367x over previous
"""AxialAttention Trainium2 kernel (8 NeuronCores, SPMD).

Sharding: core = b*4 + q; each core handles one batch element and a 10-row
H-slab with all 256 channels. The three reference "branches" are numerically
identical (h=w=d=40), so out = 3 * branch; the 3 is folded into wp and the
attention scale into wq/bq.

v3: all q/k/v pivot transposes run on the DMA engines' XBAR transpose
(InstDmaTransposeAnt) instead of the PE array, eliminating both the PE
transpose cost and the psum->SBUF evacuation traffic of the pivots.

Per H-slice, q/k are stored padded as [c, (w, deck*64 + d)] and v as
[c, (d, deck*64 + w)] (128-column chunks; 64-pitch pad). The XBAR transpose
semantics   out[p, 128j + c] = in[c, 128j + p]   then land deck-0 data at
partitions 0-39 and deck-1 at 64-103 -- both legal matmul partition bases.
Scores/AV run per (channel, deck) on 40x40 tiles read from the transposed
tiles via strided APs; softmax denominators come from a persistent ones
column appended to v (chunk 40 of each transposed slot).

conv/wp/AV psum tiles are double-wide ([128, 1024] f32 spanning 2 banks;
each matmul still targets a single bank) so one engine op drains two banks,
halving fixed per-op evacuation overheads.
"""

import sys

sys.path.insert(0, "/opt/trn_rl_repo")

import numpy as np
import ml_dtypes
from contextlib import ExitStack

import concourse.bass as bass
import concourse.tile as tile
from concourse import bacc, mybir
from concourse.bass_utils import run_bass_kernel_spmd
from concourse.masks import make_identity

BF16 = mybir.dt.bfloat16
F32 = mybir.dt.float32

B, C, H, W, D = 2, 256, 40, 40, 40
HEADS = 8
HD = C // HEADS
SCALE = HD ** -0.5
N_CORES = 8
SLAB = H // 4           # 10 H-rows per core
WD = W * D              # 1600
NSLAB = SLAB * WD       # 16000
PADC = W * 128          # 5120 padded cols per tensor per slice
GRP = 12                # channels per psum bank group
NGRP = (128 + GRP - 1) // GRP   # 11


def _merge(a, b):
    """Proportionally interleave two chunk lists, preserving each order."""
    out = []
    na, nb = len(a), len(b)
    ia = ib = 0
    while ia < na or ib < nb:
        if ib >= nb or (ia * (nb + 1) <= ib * (na + 1) and ia < na):
            out.append(a[ia])
            ia += 1
        else:
            out.append(b[ib])
            ib += 1
    return out


def _build_nc():
    nc = bacc.Bacc(
        "TRN2",
        target_bir_lowering=False,
        debug=False,
        num_devices=N_CORES,
    )
    x_d = nc.declare_dram_parameter("x", [C, NSLAB], BF16, isOutput=False)
    wqkv_d = nc.declare_dram_parameter("wqkv", [C, 3 * C], BF16, isOutput=False)
    bqkv_d = nc.declare_dram_parameter("bqkv", [3 * C, 1], F32, isOutput=False)
    wp_d = nc.declare_dram_parameter("wp3", [C, C], BF16, isOutput=False)
    bp_d = nc.declare_dram_parameter("bp", [C, 1], F32, isOutput=False)
    out_d = nc.declare_dram_parameter("out", [C, NSLAB], F32, isOutput=True)

    IDENT = mybir.ActivationFunctionType.Identity
    EXP = mybir.ActivationFunctionType.Exp
    MULT = mybir.AluOpType.mult

    with ExitStack() as ctx:
        tc = ctx.enter_context(tile.TileContext(nc))
        const = ctx.enter_context(tc.tile_pool(name="const", bufs=1))
        xp = ctx.enter_context(tc.tile_pool(name="xp", bufs=2))
        padp = ctx.enter_context(tc.tile_pool(name="padp", bufs=2))
        ttp = ctx.enter_context(tc.tile_pool(name="ttp", bufs=8))
        oap = ctx.enter_context(tc.tile_pool(name="oap", bufs=2))
        brp = ctx.enter_context(tc.tile_pool(name="brp", bufs=1))
        ep = ctx.enter_context(tc.tile_pool(name="ep", bufs=3))
        recp = ctx.enter_context(tc.tile_pool(name="recp", bufs=4))
        # conv/scores/wp/pivot-back share one 4-deep 2KB tag; AV has its own
        ps_a = ctx.enter_context(tc.tile_pool(name="ps_a", bufs=4, space="PSUM"))
        ps_o = ctx.enter_context(tc.tile_pool(name="ps_o", bufs=2, space="PSUM"))
        ps_t = ctx.enter_context(tc.tile_pool(name="ps_t", bufs=2, space="PSUM"))

        ident = const.tile([128, 128], BF16)
        make_identity(nc, ident[:])

        wqkv_sb = const.tile([128, 2, 3 * C], BF16)
        nc.sync.dma_start(
            wqkv_sb[:], wqkv_d.ap().rearrange("(ko ki) m -> ki ko m", ki=128)
        )
        wp_sb = const.tile([128, 2, C], BF16)
        nc.sync.dma_start(
            wp_sb[:], wp_d.ap().rearrange("(ko ki) m -> ki ko m", ki=128)
        )
        bqkv_sb = const.tile([128, 6, 1], F32)
        nc.sync.dma_start(
            bqkv_sb[:], bqkv_d.ap().rearrange("(mo mi) one -> mi mo one", mi=128)
        )
        bp_sb = const.tile([128, 2, 1], F32)
        nc.sync.dma_start(
            bp_sb[:], bp_d.ap().rearrange("(mo mi) one -> mi mo one", mi=128)
        )

        branch = brp.tile([128, 2, WD], BF16)
        out_h = brp.tile([128, 2, 800], F32, name="out_h")

        def load_x(i):
            x_sb = xp.tile([128, 2, WD], BF16, name="x_sb")
            nc.sync.dma_start(
                x_sb[:],
                x_d.ap()[:, i * WD : (i + 1) * WD].rearrange(
                    "(ko ki) n -> ki ko n", ki=128
                ),
            )
            return x_sb

        def tslot():
            # [128, 41, 128]; chunk 40 is the persistent ones column (set once
            # below; transpose DMAs only ever write chunks 0-39).
            return ttp.tile([128, 41, 128], BF16, tag="tt", name="tslot")

        evac_rr = [0]

        def evac(dst, src, bias=None):
            r = evac_rr[0] % 2
            evac_rr[0] += 1
            if bias is None:
                if r == 0:
                    nc.vector.tensor_copy(out=dst, in_=src)
                else:
                    nc.scalar.copy(dst, src)
            else:
                if r == 0:
                    nc.vector.tensor_scalar_add(dst, src, bias)
                else:
                    nc.scalar.activation(
                        out=dst, in_=src, func=IDENT, bias=bias, scale=1.0
                    )

        def conv_emit(xref, slots):
            """qkv conv of one slice into padded q/k/v, one transpose DMA per
            tensor. Double-wide chunks: (m-block, n-pair) -> 2 psum banks ->
            one evacuation of [128, 2x400]. xref is a 1-elem list holding the
            x tile (filled by the preceding load chunk)."""
            chunks = []
            pads = [None, None, None]

            for tn in range(3):
                def alloc_pad(tn=tn):
                    pads[tn] = padp.tile(
                        [128, PADC], BF16, tag=f"pad{tn}", name=f"pad{tn}"
                    )
                for deck in range(2):
                    m = 2 * tn + deck
                    for n in range(4):
                        def ch(tn=tn, deck=deck, m=m, n=n, alloc_pad=alloc_pad):
                            if pads[tn] is None:
                                alloc_pad()
                            pad = pads[tn]
                            ps = ps_a.tile(
                                [128, 512], F32, tag="ps_a", name="conv_ps"
                            )[:, 0:400]
                            for k in range(2):
                                nc.tensor.matmul(
                                    ps[:],
                                    lhsT=wqkv_sb[:, k, m * 128 : (m + 1) * 128],
                                    rhs=xref[0][:, k, n * 400 : (n + 1) * 400],
                                    start=(k == 0),
                                    stop=(k == 1),
                                )
                            w0 = 10 * n
                            ps_v = ps.rearrange("p (w d) -> p w d", d=40)
                            if tn < 2:
                                # q/k: col = w*128 + deck*64 + d
                                dst = pad.rearrange(
                                    "p (w x) -> p w x", x=128
                                )[:, w0 : w0 + 10, 64 * deck : 64 * deck + 40]
                            else:
                                # v: col = d*128 + deck*64 + w
                                dst = pad.rearrange(
                                    "p (d x) -> p x d", x=128
                                )[:, 64 * deck + w0 : 64 * deck + w0 + 10, :]
                            evac(dst, ps_v, bqkv_sb[:, m])
                        chunks.append(ch)

                def dma_ch(tn=tn):
                    t = tslot()
                    slots[tn] = t
                    nc.sync.dma_start(t[:, 0:40, :], pads[tn][:], transpose=True)
                chunks.append(dma_ch)
            return chunks

        def attn_emit(slots):
            """scores -> exp -> AV per 12-channel group; AV psum double-wide,
            evacuated per group-pair into o_all."""
            o_all = oap.tile([128, 128 * W], BF16, name="o_all")

            def scores_stage(g):
                c0 = g * GRP
                gn = min(GRP, 128 - c0)
                s_ps = ps_a.tile([128, 512], F32, tag="ps_a", name="s_ps")
                for j in range(gn):
                    c = c0 + j
                    for dk in range(2):
                        r = 64 * dk
                        nc.tensor.matmul(
                            s_ps[r : r + 40, j * 40 : (j + 1) * 40],
                            lhsT=slots[1][r : r + 40, 0:40, c],
                            rhs=slots[0][r : r + 40, 0:40, c],
                            start=True,
                            stop=True,
                        )
                e_sb = ep.tile([128, 480], BF16, tag="e_sb", name="e_sb")
                nc.scalar.activation(
                    out=e_sb[0:104, : gn * 40],
                    in_=s_ps[0:104, : gn * 40],
                    func=EXP,
                )
                return e_sb

            def av_stage(g, e_sb):
                c0 = g * GRP
                gn = min(GRP, 128 - c0)
                o_ps = ps_o.tile([128, 512], F32, tag="ps_o", name="o_ps")
                for j in range(gn):
                    c = c0 + j
                    for dk in range(2):
                        r = 64 * dk
                        nc.tensor.matmul(
                            o_ps[r : r + 41, j * 40 : (j + 1) * 40],
                            lhsT=slots[2][r : r + 40, 0:41, c],
                            rhs=e_sb[r : r + 40, j * 40 : (j + 1) * 40],
                            start=True,
                            stop=True,
                        )
                evac(
                    o_all[0:105, c0 * 40 : (c0 + gn) * 40],
                    o_ps[0:105, : gn * 40],
                )

            pend = [None]
            chunks = []
            for g in range(NGRP):
                def ch(g=g):
                    e_sb = scores_stage(g)
                    if pend[0] is not None:
                        av_stage(*pend[0])
                    pend[0] = (g, e_sb)
                chunks.append(ch)
            chunks.append(lambda: av_stage(*pend[0]))
            return o_all, chunks

        def pbwp_emit(o_all, i):
            """pivot-back + normalize + wp conv + out DMA for slice i."""
            o_v = o_all.rearrange("p (c w) -> p w c", w=W)
            chunks = []
            for wb in range(5):
                def ch(wb=wb):
                    w0 = 8 * wb
                    pb_full = ps_t.tile([128, 1024], BF16, tag="ps_t", name="pb")
                    pb = pb_full[:, 0:848]
                    for wl in range(8):
                        w = w0 + wl
                        nc.tensor.transpose(
                            pb[:, wl * 106 : wl * 106 + 105],
                            o_v[0:105, w, :],
                            ident[0:105, 0:105],
                        )
                    pb_v = pb.rearrange("p (w q) -> p w q", q=106)
                    rec = recp.tile([128, 8, 2], F32, tag="rec", name="rec")
                    nc.vector.reciprocal(rec[:, :, 0], pb_v[:, :, 40])
                    nc.vector.reciprocal(rec[:, :, 1], pb_v[:, :, 104])
                    for dk in range(2):
                        nc.vector.tensor_tensor(
                            branch[:, dk].rearrange("p (w d) -> p w d", d=40)[
                                :, w0 : w0 + 8
                            ],
                            pb_v[:, :, 64 * dk : 64 * dk + 40],
                            rec[:, :, dk : dk + 1].to_broadcast((128, 8, 40)),
                            MULT,
                        )
                chunks.append(ch)
            # wp: half-slice granularity; psum double-wide over (n-pair)
            for hs in range(2):
                for nl in range(2):
                    for m in range(2):
                        def ch(hs=hs, nl=nl, m=m):
                            n = 2 * hs + nl
                            ps = ps_a.tile(
                                [128, 512], F32, tag="ps_a", name="wp_ps"
                            )[:, 0:400]
                            for k in range(2):
                                nc.tensor.matmul(
                                    ps[:],
                                    lhsT=wp_sb[:, k, m * 128 : (m + 1) * 128],
                                    rhs=branch[:, k, n * 400 : (n + 1) * 400],
                                    start=(k == 0),
                                    stop=(k == 1),
                                )
                            evac(
                                out_h[:, m, nl * 400 : (nl + 1) * 400],
                                ps[:],
                                bp_sb[:, m],
                            )
                        chunks.append(ch)

                def dma_ch(hs=hs, i=i):
                    nc.sync.dma_start(
                        out_d.ap()[
                            :, i * WD + hs * 800 : i * WD + (hs + 1) * 800
                        ].rearrange("(ko ki) n -> ki ko n", ki=128),
                        out_h[:],
                    )
                chunks.append(dma_ch)
            return chunks

        # ones columns for AV denominators: chunk 40 of each of the 8 slots.
        first = [tslot() for _ in range(8)]
        for t in first:
            nc.vector.memset(t[:, 40, :], 1.0)
        del first  # pool rotation reuses these slots

        # ---- software-pipelined slice loop
        xref = [load_x(0)]
        slots_cur = [None, None, None]
        for ch in conv_emit(xref, slots_cur):
            ch()
        pending_pbwp = []
        for i in range(SLAB):
            o_all, attn_ch = attn_emit(slots_cur)
            if i + 1 < SLAB:
                xref_n = [None]
                def ch_load(i=i, xref_n=xref_n):
                    xref_n[0] = load_x(i + 1)
                slots_nxt = [None, None, None]
                conv_chunks = [ch_load] + conv_emit(xref_n, slots_nxt)
                other = _merge(pending_pbwp, conv_chunks)
            else:
                slots_nxt, other = None, pending_pbwp
            for ch in _merge(attn_ch, other):
                ch()
            pending_pbwp = pbwp_emit(o_all, i)
            slots_cur = slots_nxt
        for ch in pending_pbwp:
            ch()

    nc.compile()
    return nc


_NC_CACHE = None


def _get_nc():
    global _NC_CACHE
    if _NC_CACHE is None:
        _NC_CACHE = _build_nc()
    return _NC_CACHE


def make_in_maps(x, wq, bq, wk, bk, wv, bv, wp, bp):
    bf = ml_dtypes.bfloat16
    wqkv = np.concatenate(
        [wq.T * SCALE, wk.T, wv.T], axis=1
    ).astype(bf)  # [C, 3C], lhsT layout (c_in rows, c_out cols)
    bqkv = np.concatenate([bq * SCALE, bk, bv]).reshape(3 * C, 1).astype(np.float32)
    wp3 = (3.0 * wp).T.astype(bf)  # [C, C]
    bp_ = bp.reshape(C, 1).astype(np.float32)
    in_maps = []
    for core in range(N_CORES):
        b = core // 4
        r0 = (core % 4) * SLAB
        x_slab = np.ascontiguousarray(
            x[b, :, r0 : r0 + SLAB].reshape(C, NSLAB)
        ).astype(bf)
        in_maps.append(
            {"x": x_slab, "wqkv": wqkv, "bqkv": bqkv, "wp3": wp3, "bp": bp_}
        )
    return in_maps


def run_on_cores(in_maps, **kw):
    nc = _get_nc()
    return run_bass_kernel_spmd(nc, in_maps, core_ids=list(range(N_CORES)), **kw)


def kernel(x, wq, bq, wk, bk, wv, bv, wp, bp):
    x = np.asarray(x, dtype=np.float32)
    in_maps = make_in_maps(
        x,
        np.asarray(wq, np.float32),
        np.asarray(bq, np.float32),
        np.asarray(wk, np.float32),
        np.asarray(bk, np.float32),
        np.asarray(wv, np.float32),
        np.asarray(bv, np.float32),
        np.asarray(wp, np.float32),
        np.asarray(bp, np.float32),
    )
    res = run_on_cores(in_maps)
    out = np.empty((B, C, H, W, D), np.float32)
    for core in range(N_CORES):
        b = core // 4
        r0 = (core % 4) * SLAB
        out[b, :, r0 : r0 + SLAB] = res.results[core]["out"].reshape(C, SLAB, W, D)
    return out


if __name__ == "__main__":
    rng = np.random.default_rng(0)
    ins = {
        "x": rng.standard_normal((B, C, H, W, D), np.float32),
        "wq": rng.standard_normal((C, C), np.float32) / 16,
        "bq": rng.standard_normal(C).astype(np.float32) * 0.01,
        "wk": rng.standard_normal((C, C), np.float32) / 16,
        "bk": rng.standard_normal(C).astype(np.float32) * 0.01,
        "wv": rng.standard_normal((C, C), np.float32) / 16,
        "bv": rng.standard_normal(C).astype(np.float32) * 0.01,
        "wp": rng.standard_normal((C, C), np.float32) / 16,
        "bp": rng.standard_normal(C).astype(np.float32) * 0.01,
    }
    out = kernel(**ins)
    print("kernel ran, out shape", out.shape, "mean", float(np.abs(out).mean()))


# revision 13
# speedup vs baseline: 1.1860x; 1.0323x over previous
"""AxialAttention Trainium2 kernel (8 NeuronCores, SPMD).

Sharding: core = b*4 + q; each core handles one batch element and a 10-row
H-slab with all 256 channels. The three reference "branches" are numerically
identical (h=w=d=40), so out = 3 * branch; the 3 is folded into wp and the
attention scale into wq/bq.

v3: all q/k/v pivot transposes run on the DMA engines' XBAR transpose
(InstDmaTransposeAnt) instead of the PE array, eliminating both the PE
transpose cost and the psum->SBUF evacuation traffic of the pivots.

Per H-slice, q/k are stored padded as [c, (w, deck*64 + d)] and v as
[c, (d, deck*64 + w)] (128-column chunks; 64-pitch pad). The XBAR transpose
semantics   out[p, 128j + c] = in[c, 128j + p]   then land deck-0 data at
partitions 0-39 and deck-1 at 64-103 -- both legal matmul partition bases.
Scores/AV run per (channel, deck) on 40x40 tiles read from the transposed
tiles via strided APs; softmax denominators come from a persistent ones
column appended to v (chunk 40 of each transposed slot).

conv/wp/AV psum tiles are double-wide ([128, 1024] f32 spanning 2 banks;
each matmul still targets a single bank) so one engine op drains two banks,
halving fixed per-op evacuation overheads.
"""

import sys

sys.path.insert(0, "/opt/trn_rl_repo")

import numpy as np
import ml_dtypes
from contextlib import ExitStack

import concourse.bass as bass
import concourse.tile as tile
from concourse import bacc, mybir
from concourse.bass_utils import run_bass_kernel_spmd
from concourse.masks import make_identity

BF16 = mybir.dt.bfloat16
F32 = mybir.dt.float32

B, C, H, W, D = 2, 256, 40, 40, 40
HEADS = 8
HD = C // HEADS
SCALE = HD ** -0.5
N_CORES = 8
SLAB = H // 4           # 10 H-rows per core
WD = W * D              # 1600
NSLAB = SLAB * WD       # 16000
PADC = W * 128          # 5120 padded cols per tensor per slice
GRP = 12                # channels per psum bank group
NGRP = (128 + GRP - 1) // GRP   # 11


def _merge(a, b):
    """Proportionally interleave two chunk lists, preserving each order."""
    out = []
    na, nb = len(a), len(b)
    ia = ib = 0
    while ia < na or ib < nb:
        if ib >= nb or (ia * (nb + 1) <= ib * (na + 1) and ia < na):
            out.append(a[ia])
            ia += 1
        else:
            out.append(b[ib])
            ib += 1
    return out


def _build_nc():
    nc = bacc.Bacc(
        "TRN2",
        target_bir_lowering=False,
        debug=False,
        num_devices=N_CORES,
    )
    x_d = nc.declare_dram_parameter("x", [C, NSLAB], BF16, isOutput=False)
    wqkv_d = nc.declare_dram_parameter("wqkv", [C, 3 * C], BF16, isOutput=False)
    bqkv_d = nc.declare_dram_parameter("bqkv", [3 * C, 1], F32, isOutput=False)
    wp_d = nc.declare_dram_parameter("wp3", [C, C], BF16, isOutput=False)
    bp_d = nc.declare_dram_parameter("bp", [C, 1], F32, isOutput=False)
    out_d = nc.declare_dram_parameter("out", [C, NSLAB], F32, isOutput=True)

    IDENT = mybir.ActivationFunctionType.Identity
    EXP = mybir.ActivationFunctionType.Exp
    MULT = mybir.AluOpType.mult

    with ExitStack() as ctx:
        tc = ctx.enter_context(tile.TileContext(nc))
        const = ctx.enter_context(tc.tile_pool(name="const", bufs=1))
        xp = ctx.enter_context(tc.tile_pool(name="xp", bufs=2))
        padp = ctx.enter_context(tc.tile_pool(name="padp", bufs=2))
        ttp = ctx.enter_context(tc.tile_pool(name="ttp", bufs=8))
        oap = ctx.enter_context(tc.tile_pool(name="oap", bufs=2))
        brp = ctx.enter_context(tc.tile_pool(name="brp", bufs=1))
        ep = ctx.enter_context(tc.tile_pool(name="ep", bufs=3))
        recp = ctx.enter_context(tc.tile_pool(name="recp", bufs=4))
        # conv/scores/wp/pivot-back share one 4-deep 2KB tag; AV has its own
        ps_a = ctx.enter_context(tc.tile_pool(name="ps_a", bufs=4, space="PSUM"))
        ps_o = ctx.enter_context(tc.tile_pool(name="ps_o", bufs=2, space="PSUM"))
        ps_t = ctx.enter_context(tc.tile_pool(name="ps_t", bufs=2, space="PSUM"))

        ident = const.tile([128, 128], BF16)
        make_identity(nc, ident[:])

        wqkv_sb = const.tile([128, 2, 3 * C], BF16)
        nc.sync.dma_start(
            wqkv_sb[:], wqkv_d.ap().rearrange("(ko ki) m -> ki ko m", ki=128)
        )
        wp_sb = const.tile([128, 2, C], BF16)
        nc.sync.dma_start(
            wp_sb[:], wp_d.ap().rearrange("(ko ki) m -> ki ko m", ki=128)
        )
        bqkv_sb = const.tile([128, 6, 1], F32)
        nc.sync.dma_start(
            bqkv_sb[:], bqkv_d.ap().rearrange("(mo mi) one -> mi mo one", mi=128)
        )
        bp_sb = const.tile([128, 2, 1], F32)
        nc.sync.dma_start(
            bp_sb[:], bp_d.ap().rearrange("(mo mi) one -> mi mo one", mi=128)
        )

        branch = brp.tile([128, 2, WD], BF16)
        out_h = brp.tile([128, 2, 800], F32, name="out_h")

        def load_x(i):
            x_sb = xp.tile([128, 2, WD], BF16, name="x_sb")
            nc.sync.dma_start(
                x_sb[:],
                x_d.ap()[:, i * WD : (i + 1) * WD].rearrange(
                    "(ko ki) n -> ki ko n", ki=128
                ),
            )
            return x_sb

        def tslot():
            # [128, 41, 128]; chunk 40 is the persistent ones column (set once
            # below; transpose DMAs only ever write chunks 0-39).
            return ttp.tile([128, 41, 128], BF16, tag="tt", name="tslot")

        evac_rr = [0]

        def evac(dst, src, bias=None):
            r = evac_rr[0] % 2
            evac_rr[0] += 1
            if bias is None:
                if r == 0:
                    nc.vector.tensor_copy(out=dst, in_=src)
                else:
                    nc.scalar.copy(dst, src)
            else:
                if r == 0:
                    nc.vector.tensor_scalar_add(dst, src, bias)
                else:
                    nc.scalar.activation(
                        out=dst, in_=src, func=IDENT, bias=bias, scale=1.0
                    )

        def conv_emit(xref, slots):
            """qkv conv of one slice into padded q/k/v, one transpose DMA per
            tensor. Double-wide chunks: (m-block, n-pair) -> 2 psum banks ->
            one evacuation of [128, 2x400]. xref is a 1-elem list holding the
            x tile (filled by the preceding load chunk)."""
            chunks = []
            pads = [None, None, None]

            for tn in range(3):
                def alloc_pad(tn=tn):
                    pads[tn] = padp.tile(
                        [128, PADC], BF16, tag=f"pad{tn}", name=f"pad{tn}"
                    )
                for deck in range(2):
                    m = 2 * tn + deck
                    for n in range(4):
                        def ch(tn=tn, deck=deck, m=m, n=n, alloc_pad=alloc_pad):
                            if pads[tn] is None:
                                alloc_pad()
                            pad = pads[tn]
                            ps = ps_a.tile(
                                [128, 512], F32, tag="ps_a", name="conv_ps"
                            )[:, 0:400]
                            for k in range(2):
                                nc.tensor.matmul(
                                    ps[:],
                                    lhsT=wqkv_sb[:, k, m * 128 : (m + 1) * 128],
                                    rhs=xref[0][:, k, n * 400 : (n + 1) * 400],
                                    start=(k == 0),
                                    stop=(k == 1),
                                )
                            w0 = 10 * n
                            ps_v = ps.rearrange("p (w d) -> p w d", d=40)
                            if tn < 2:
                                # q/k: col = w*128 + deck*64 + d
                                dst = pad.rearrange(
                                    "p (w x) -> p w x", x=128
                                )[:, w0 : w0 + 10, 64 * deck : 64 * deck + 40]
                            else:
                                # v: col = d*128 + deck*64 + w
                                dst = pad.rearrange(
                                    "p (d x) -> p x d", x=128
                                )[:, 64 * deck + w0 : 64 * deck + w0 + 10, :]
                            evac(dst, ps_v, bqkv_sb[:, m])
                        chunks.append(ch)

                def dma_ch(tn=tn):
                    t = tslot()
                    slots[tn] = t
                    nc.sync.dma_start(t[:, 0:40, :], pads[tn][:], transpose=True)
                chunks.append(dma_ch)
            return chunks

        def attn_emit(slots):
            """scores -> exp -> AV per 12-channel group; AV psum double-wide,
            evacuated per group-pair into o_all."""
            o_all = oap.tile([128, 128 * W], BF16, name="o_all")

            def scores_stage(g):
                c0 = g * GRP
                gn = min(GRP, 128 - c0)
                s_ps = ps_a.tile([128, 512], F32, tag="ps_a", name="s_ps")
                for j in range(gn):
                    c = c0 + j
                    for dk in range(2):
                        r = 64 * dk
                        nc.tensor.matmul(
                            s_ps[r : r + 40, j * 40 : (j + 1) * 40],
                            lhsT=slots[1][r : r + 40, 0:40, c],
                            rhs=slots[0][r : r + 40, 0:40, c],
                            start=True,
                            stop=True,
                        )
                e_sb = ep.tile([128, 480], BF16, tag="e_sb", name="e_sb")
                nc.scalar.activation(
                    out=e_sb[0:104, : gn * 40],
                    in_=s_ps[0:104, : gn * 40],
                    func=EXP,
                )
                return e_sb

            def av_stage(g, e_sb):
                c0 = g * GRP
                gn = min(GRP, 128 - c0)
                o_ps = ps_o.tile([128, 512], F32, tag="ps_o", name="o_ps")
                for j in range(gn):
                    c = c0 + j
                    for dk in range(2):
                        r = 64 * dk
                        nc.tensor.matmul(
                            o_ps[r : r + 41, j * 40 : (j + 1) * 40],
                            lhsT=slots[2][r : r + 40, 0:41, c],
                            rhs=e_sb[r : r + 40, j * 40 : (j + 1) * 40],
                            start=True,
                            stop=True,
                        )
                evac(
                    o_all[0:105, c0 * 40 : (c0 + gn) * 40],
                    o_ps[0:105, : gn * 40],
                )

            pend = [None]
            chunks = []
            for g in range(NGRP):
                def ch(g=g):
                    e_sb = scores_stage(g)
                    if pend[0] is not None:
                        av_stage(*pend[0])
                    pend[0] = (g, e_sb)
                chunks.append(ch)
            chunks.append(lambda: av_stage(*pend[0]))
            return o_all, chunks

        def pbwp_emit(o_all, i):
            """pivot-back + normalize + wp conv + out DMA for slice i."""
            o_v = o_all.rearrange("p (c w) -> p w c", w=W)
            chunks = []
            for wb in range(5):
                def ch(wb=wb):
                    w0 = 8 * wb
                    pb_full = ps_t.tile([128, 1024], BF16, tag="ps_t", name="pb")
                    pb = pb_full[:, 0:848]
                    for wl in range(8):
                        w = w0 + wl
                        nc.tensor.transpose(
                            pb[:, wl * 106 : wl * 106 + 105],
                            o_v[0:105, w, :],
                            ident[0:105, 0:105],
                        )
                    pb_v = pb.rearrange("p (w q) -> p w q", q=106)
                    rec = recp.tile([128, 8, 2], F32, tag="rec", name="rec")
                    nc.vector.reciprocal(rec[:, :, 0], pb_v[:, :, 40])
                    nc.vector.reciprocal(rec[:, :, 1], pb_v[:, :, 104])
                    for dk in range(2):
                        nc.vector.tensor_tensor(
                            branch[:, dk].rearrange("p (w d) -> p w d", d=40)[
                                :, w0 : w0 + 8
                            ],
                            pb_v[:, :, 64 * dk : 64 * dk + 40],
                            rec[:, :, dk : dk + 1].to_broadcast((128, 8, 40)),
                            MULT,
                        )
                chunks.append(ch)
            # wp: half-slice granularity; psum double-wide over (n-pair)
            for hs in range(2):
                for nl in range(2):
                    for m in range(2):
                        def ch(hs=hs, nl=nl, m=m):
                            n = 2 * hs + nl
                            ps = ps_a.tile(
                                [128, 512], F32, tag="ps_a", name="wp_ps"
                            )[:, 0:400]
                            for k in range(2):
                                nc.tensor.matmul(
                                    ps[:],
                                    lhsT=wp_sb[:, k, m * 128 : (m + 1) * 128],
                                    rhs=branch[:, k, n * 400 : (n + 1) * 400],
                                    start=(k == 0),
                                    stop=(k == 1),
                                )
                            evac(
                                out_h[:, m, nl * 400 : (nl + 1) * 400],
                                ps[:],
                                bp_sb[:, m],
                            )
                        chunks.append(ch)

                def dma_ch(hs=hs, i=i):
                    nc.sync.dma_start(
                        out_d.ap()[
                            :, i * WD + hs * 800 : i * WD + (hs + 1) * 800
                        ].rearrange("(ko ki) n -> ki ko n", ki=128),
                        out_h[:],
                    )
                chunks.append(dma_ch)
            return chunks

        # ones columns for AV denominators: chunk 40 of each of the 8 slots.
        first = [tslot() for _ in range(8)]
        for t in first:
            nc.vector.memset(t[:, 40, :], 1.0)
        del first  # pool rotation reuses these slots

        # ---- software-pipelined slice loop (3 stages deep)
        # conv runs TWO slices ahead of attention so the transpose DMAs of
        # slice i complete long before scores(i) need them.
        xref0 = [load_x(0)]
        slots = {0: [None, None, None]}
        for ch in conv_emit(xref0, slots[0]):
            ch()
        if SLAB > 1:
            xref1 = [load_x(1)]
            slots[1] = [None, None, None]
            for ch in conv_emit(xref1, slots[1]):
                ch()
        pending_pbwp = []
        for i in range(SLAB):
            o_all, attn_ch = attn_emit(slots.pop(i))
            if i + 2 < SLAB:
                xref_n = [None]
                def ch_load(i=i, xref_n=xref_n):
                    xref_n[0] = load_x(i + 2)
                slots[i + 2] = [None, None, None]
                conv_chunks = [ch_load] + conv_emit(xref_n, slots[i + 2])
                other = _merge(pending_pbwp, conv_chunks)
            else:
                other = pending_pbwp
            for ch in _merge(attn_ch, other):
                ch()
            pending_pbwp = pbwp_emit(o_all, i)
        for ch in pending_pbwp:
            ch()

    nc.compile()
    return nc


_NC_CACHE = None


def _get_nc():
    global _NC_CACHE
    if _NC_CACHE is None:
        _NC_CACHE = _build_nc()
    return _NC_CACHE


def make_in_maps(x, wq, bq, wk, bk, wv, bv, wp, bp):
    bf = ml_dtypes.bfloat16
    wqkv = np.concatenate(
        [wq.T * SCALE, wk.T, wv.T], axis=1
    ).astype(bf)  # [C, 3C], lhsT layout (c_in rows, c_out cols)
    bqkv = np.concatenate([bq * SCALE, bk, bv]).reshape(3 * C, 1).astype(np.float32)
    wp3 = (3.0 * wp).T.astype(bf)  # [C, C]
    bp_ = bp.reshape(C, 1).astype(np.float32)
    in_maps = []
    for core in range(N_CORES):
        b = core // 4
        r0 = (core % 4) * SLAB
        x_slab = np.ascontiguousarray(
            x[b, :, r0 : r0 + SLAB].reshape(C, NSLAB)
        ).astype(bf)
        in_maps.append(
            {"x": x_slab, "wqkv": wqkv, "bqkv": bqkv, "wp3": wp3, "bp": bp_}
        )
    return in_maps


def run_on_cores(in_maps, **kw):
    nc = _get_nc()
    return run_bass_kernel_spmd(nc, in_maps, core_ids=list(range(N_CORES)), **kw)


def kernel(x, wq, bq, wk, bk, wv, bv, wp, bp):
    x = np.asarray(x, dtype=np.float32)
    in_maps = make_in_maps(
        x,
        np.asarray(wq, np.float32),
        np.asarray(bq, np.float32),
        np.asarray(wk, np.float32),
        np.asarray(bk, np.float32),
        np.asarray(wv, np.float32),
        np.asarray(bv, np.float32),
        np.asarray(wp, np.float32),
        np.asarray(bp, np.float32),
    )
    res = run_on_cores(in_maps)
    out = np.empty((B, C, H, W, D), np.float32)
    for core in range(N_CORES):
        b = core // 4
        r0 = (core % 4) * SLAB
        out[b, :, r0 : r0 + SLAB] = res.results[core]["out"].reshape(C, SLAB, W, D)
    return out


if __name__ == "__main__":
    rng = np.random.default_rng(0)
    ins = {
        "x": rng.standard_normal((B, C, H, W, D), np.float32),
        "wq": rng.standard_normal((C, C), np.float32) / 16,
        "bq": rng.standard_normal(C).astype(np.float32) * 0.01,
        "wk": rng.standard_normal((C, C), np.float32) / 16,
        "bk": rng.standard_normal(C).astype(np.float32) * 0.01,
        "wv": rng.standard_normal((C, C), np.float32) / 16,
        "bv": rng.standard_normal(C).astype(np.float32) * 0.01,
        "wp": rng.standard_normal((C, C), np.float32) / 16,
        "bp": rng.standard_normal(C).astype(np.float32) * 0.01,
    }
    out = kernel(**ins)
    print("kernel ran, out shape", out.shape, "mean", float(np.abs(out).mean()))


# revision 14
# speedup vs baseline: 1.2443x; 1.0491x over previous
"""AxialAttention Trainium2 kernel (8 NeuronCores, SPMD).

Sharding: core = b*4 + q; each core handles one batch element and a 10-row
H-slab with all 256 channels. The three reference "branches" are numerically
identical (h=w=d=40), so out = 3 * branch; the 3 is folded into wp and the
attention scale into wq/bq.

v3: all q/k/v pivot transposes run on the DMA engines' XBAR transpose
(InstDmaTransposeAnt) instead of the PE array, eliminating both the PE
transpose cost and the psum->SBUF evacuation traffic of the pivots.

Per H-slice, q/k are stored padded as [c, (w, deck*64 + d)] and v as
[c, (d, deck*64 + w)] (128-column chunks; 64-pitch pad). The XBAR transpose
semantics   out[p, 128j + c] = in[c, 128j + p]   then land deck-0 data at
partitions 0-39 and deck-1 at 64-103 -- both legal matmul partition bases.
Scores/AV run per (channel, deck) on 40x40 tiles read from the transposed
tiles via strided APs; softmax denominators come from a persistent ones
column appended to v (chunk 40 of each transposed slot).

conv/wp/AV psum tiles are double-wide ([128, 1024] f32 spanning 2 banks;
each matmul still targets a single bank) so one engine op drains two banks,
halving fixed per-op evacuation overheads.
"""

import sys

sys.path.insert(0, "/opt/trn_rl_repo")

import numpy as np
import ml_dtypes
from contextlib import ExitStack

import concourse.bass as bass
import concourse.tile as tile
from concourse import bacc, mybir
from concourse.bass_utils import run_bass_kernel_spmd
from concourse.masks import make_identity

BF16 = mybir.dt.bfloat16
F32 = mybir.dt.float32

B, C, H, W, D = 2, 256, 40, 40, 40
HEADS = 8
HD = C // HEADS
SCALE = HD ** -0.5
N_CORES = 8
SLAB = H // 4           # 10 H-rows per core
WD = W * D              # 1600
NSLAB = SLAB * WD       # 16000
PADC = W * 128          # 5120 padded cols per tensor per slice
GRP = 12                # channels per psum bank group
NGRP = (128 + GRP - 1) // GRP   # 11


def _merge(a, b):
    """Proportionally interleave two chunk lists, preserving each order."""
    out = []
    na, nb = len(a), len(b)
    ia = ib = 0
    while ia < na or ib < nb:
        if ib >= nb or (ia * (nb + 1) <= ib * (na + 1) and ia < na):
            out.append(a[ia])
            ia += 1
        else:
            out.append(b[ib])
            ib += 1
    return out


def _build_nc():
    nc = bacc.Bacc(
        "TRN2",
        target_bir_lowering=False,
        debug=False,
        num_devices=N_CORES,
    )
    x_d = nc.declare_dram_parameter("x", [C, NSLAB], BF16, isOutput=False)
    wqkv_d = nc.declare_dram_parameter("wqkv", [C, 3 * C], BF16, isOutput=False)
    bqkv_d = nc.declare_dram_parameter("bqkv", [3 * C, 1], F32, isOutput=False)
    wp_d = nc.declare_dram_parameter("wp3", [C, C], BF16, isOutput=False)
    bp_d = nc.declare_dram_parameter("bp", [C, 1], F32, isOutput=False)
    out_d = nc.declare_dram_parameter("out", [C, NSLAB], F32, isOutput=True)

    IDENT = mybir.ActivationFunctionType.Identity
    EXP = mybir.ActivationFunctionType.Exp
    MULT = mybir.AluOpType.mult

    with ExitStack() as ctx:
        tc = ctx.enter_context(tile.TileContext(nc))
        const = ctx.enter_context(tc.tile_pool(name="const", bufs=1))
        xp = ctx.enter_context(tc.tile_pool(name="xp", bufs=3))
        padp = ctx.enter_context(tc.tile_pool(name="padp", bufs=2))
        ttp = ctx.enter_context(tc.tile_pool(name="ttp", bufs=8))
        oap = ctx.enter_context(tc.tile_pool(name="oap", bufs=2))
        brp = ctx.enter_context(tc.tile_pool(name="brp", bufs=1))
        ep = ctx.enter_context(tc.tile_pool(name="ep", bufs=3))
        recp = ctx.enter_context(tc.tile_pool(name="recp", bufs=4))
        # conv/scores/wp/pivot-back share one 4-deep 2KB tag; AV has its own
        ps_a = ctx.enter_context(tc.tile_pool(name="ps_a", bufs=4, space="PSUM"))
        ps_o = ctx.enter_context(tc.tile_pool(name="ps_o", bufs=2, space="PSUM"))
        ps_t = ctx.enter_context(tc.tile_pool(name="ps_t", bufs=2, space="PSUM"))

        ident = const.tile([128, 128], BF16)
        make_identity(nc, ident[:])

        wqkv_sb = const.tile([128, 2, 3 * C], BF16)
        nc.sync.dma_start(
            wqkv_sb[:], wqkv_d.ap().rearrange("(ko ki) m -> ki ko m", ki=128)
        )
        wp_sb = const.tile([128, 2, C], BF16)
        nc.sync.dma_start(
            wp_sb[:], wp_d.ap().rearrange("(ko ki) m -> ki ko m", ki=128)
        )
        bqkv_sb = const.tile([128, 6, 1], F32)
        nc.sync.dma_start(
            bqkv_sb[:], bqkv_d.ap().rearrange("(mo mi) one -> mi mo one", mi=128)
        )
        bp_sb = const.tile([128, 2, 1], F32)
        nc.sync.dma_start(
            bp_sb[:], bp_d.ap().rearrange("(mo mi) one -> mi mo one", mi=128)
        )

        branch = brp.tile([128, 2, WD], BF16)
        out_h = brp.tile([128, 2, 800], F32, name="out_h")

        def load_x(i):
            x_sb = xp.tile([128, 2, WD], BF16, name="x_sb")
            nc.sync.dma_start(
                x_sb[:],
                x_d.ap()[:, i * WD : (i + 1) * WD].rearrange(
                    "(ko ki) n -> ki ko n", ki=128
                ),
            )
            return x_sb

        def tslot():
            # [128, 41, 128]; chunk 40 is the persistent ones column (set once
            # below; transpose DMAs only ever write chunks 0-39).
            return ttp.tile([128, 41, 128], BF16, tag="tt", name="tslot")

        evac_rr = [0]

        def evac(dst, src, bias=None):
            r = evac_rr[0] % 2
            evac_rr[0] += 1
            if bias is None:
                if r == 0:
                    nc.vector.tensor_copy(out=dst, in_=src)
                else:
                    nc.scalar.copy(dst, src)
            else:
                if r == 0:
                    nc.vector.tensor_scalar_add(dst, src, bias)
                else:
                    nc.scalar.activation(
                        out=dst, in_=src, func=IDENT, bias=bias, scale=1.0
                    )

        def conv_emit(xref, slots):
            """qkv conv of one slice into padded q/k/v, one transpose DMA per
            tensor. Double-wide chunks: (m-block, n-pair) -> 2 psum banks ->
            one evacuation of [128, 2x400]. xref is a 1-elem list holding the
            x tile (filled by the preceding load chunk)."""
            chunks = []
            pads = [None, None, None]

            for tn in range(3):
                def alloc_pad(tn=tn):
                    pads[tn] = padp.tile(
                        [128, PADC], BF16, tag=f"pad{tn}", name=f"pad{tn}"
                    )
                for deck in range(2):
                    m = 2 * tn + deck
                    for n in range(4):
                        def ch(tn=tn, deck=deck, m=m, n=n, alloc_pad=alloc_pad):
                            if pads[tn] is None:
                                alloc_pad()
                            pad = pads[tn]
                            ps = ps_a.tile(
                                [128, 512], F32, tag="ps_a", name="conv_ps"
                            )[:, 0:400]
                            for k in range(2):
                                nc.tensor.matmul(
                                    ps[:],
                                    lhsT=wqkv_sb[:, k, m * 128 : (m + 1) * 128],
                                    rhs=xref[0][:, k, n * 400 : (n + 1) * 400],
                                    start=(k == 0),
                                    stop=(k == 1),
                                )
                            w0 = 10 * n
                            ps_v = ps.rearrange("p (w d) -> p w d", d=40)
                            if tn < 2:
                                # q/k: col = w*128 + deck*64 + d
                                dst = pad.rearrange(
                                    "p (w x) -> p w x", x=128
                                )[:, w0 : w0 + 10, 64 * deck : 64 * deck + 40]
                            else:
                                # v: col = d*128 + deck*64 + w
                                dst = pad.rearrange(
                                    "p (d x) -> p x d", x=128
                                )[:, 64 * deck + w0 : 64 * deck + w0 + 10, :]
                            evac(dst, ps_v, bqkv_sb[:, m])
                        chunks.append(ch)

                def dma_ch(tn=tn):
                    t = tslot()
                    slots[tn] = t
                    nc.sync.dma_start(t[:, 0:40, :], pads[tn][:], transpose=True)
                chunks.append(dma_ch)
            return chunks

        def attn_emit(slots):
            """scores -> exp -> AV per 12-channel group; AV psum double-wide,
            evacuated per group-pair into o_all."""
            o_all = oap.tile([128, 128 * W], BF16, name="o_all")

            def scores_stage(g):
                c0 = g * GRP
                gn = min(GRP, 128 - c0)
                s_ps = ps_a.tile([128, 512], F32, tag="ps_a", name="s_ps")
                for j in range(gn):
                    c = c0 + j
                    for dk in range(2):
                        r = 64 * dk
                        nc.tensor.matmul(
                            s_ps[r : r + 40, j * 40 : (j + 1) * 40],
                            lhsT=slots[1][r : r + 40, 0:40, c],
                            rhs=slots[0][r : r + 40, 0:40, c],
                            start=True,
                            stop=True,
                        )
                e_sb = ep.tile([128, 480], BF16, tag="e_sb", name="e_sb")
                nc.scalar.activation(
                    out=e_sb[0:104, : gn * 40],
                    in_=s_ps[0:104, : gn * 40],
                    func=EXP,
                )
                return e_sb

            def av_stage(g, e_sb):
                c0 = g * GRP
                gn = min(GRP, 128 - c0)
                o_ps = ps_o.tile([128, 512], F32, tag="ps_o", name="o_ps")
                for j in range(gn):
                    c = c0 + j
                    for dk in range(2):
                        r = 64 * dk
                        nc.tensor.matmul(
                            o_ps[r : r + 41, j * 40 : (j + 1) * 40],
                            lhsT=slots[2][r : r + 40, 0:41, c],
                            rhs=e_sb[r : r + 40, j * 40 : (j + 1) * 40],
                            start=True,
                            stop=True,
                        )
                evac(
                    o_all[0:105, c0 * 40 : (c0 + gn) * 40],
                    o_ps[0:105, : gn * 40],
                )

            pend = [None]
            chunks = []
            for g in range(NGRP):
                def ch(g=g):
                    e_sb = scores_stage(g)
                    if pend[0] is not None:
                        av_stage(*pend[0])
                    pend[0] = (g, e_sb)
                chunks.append(ch)
            chunks.append(lambda: av_stage(*pend[0]))
            return o_all, chunks

        def pbwp_emit(o_all, i):
            """pivot-back + normalize + wp conv + out DMA for slice i."""
            o_v = o_all.rearrange("p (c w) -> p w c", w=W)
            chunks = []
            for wb in range(5):
                def ch(wb=wb):
                    w0 = 8 * wb
                    pb_full = ps_t.tile([128, 1024], BF16, tag="ps_t", name="pb")
                    pb = pb_full[:, 0:848]
                    for wl in range(8):
                        w = w0 + wl
                        nc.tensor.transpose(
                            pb[:, wl * 106 : wl * 106 + 105],
                            o_v[0:105, w, :],
                            ident[0:105, 0:105],
                        )
                    pb_v = pb.rearrange("p (w q) -> p w q", q=106)
                    rec = recp.tile([128, 8, 2], F32, tag="rec", name="rec")
                    nc.vector.reciprocal(rec[:, :, 0], pb_v[:, :, 40])
                    nc.vector.reciprocal(rec[:, :, 1], pb_v[:, :, 104])
                    for dk in range(2):
                        nc.vector.tensor_tensor(
                            branch[:, dk].rearrange("p (w d) -> p w d", d=40)[
                                :, w0 : w0 + 8
                            ],
                            pb_v[:, :, 64 * dk : 64 * dk + 40],
                            rec[:, :, dk : dk + 1].to_broadcast((128, 8, 40)),
                            MULT,
                        )
                chunks.append(ch)
            # wp: half-slice granularity; psum double-wide over (n-pair)
            for hs in range(2):
                for nl in range(2):
                    for m in range(2):
                        def ch(hs=hs, nl=nl, m=m):
                            n = 2 * hs + nl
                            ps = ps_a.tile(
                                [128, 512], F32, tag="ps_a", name="wp_ps"
                            )[:, 0:400]
                            for k in range(2):
                                nc.tensor.matmul(
                                    ps[:],
                                    lhsT=wp_sb[:, k, m * 128 : (m + 1) * 128],
                                    rhs=branch[:, k, n * 400 : (n + 1) * 400],
                                    start=(k == 0),
                                    stop=(k == 1),
                                )
                            evac(
                                out_h[:, m, nl * 400 : (nl + 1) * 400],
                                ps[:],
                                bp_sb[:, m],
                            )
                        chunks.append(ch)

                def dma_ch(hs=hs, i=i):
                    nc.sync.dma_start(
                        out_d.ap()[
                            :, i * WD + hs * 800 : i * WD + (hs + 1) * 800
                        ].rearrange("(ko ki) n -> ki ko n", ki=128),
                        out_h[:],
                    )
                chunks.append(dma_ch)
            return chunks

        # ones columns for AV denominators: chunk 40 of each of the 8 slots.
        first = [tslot() for _ in range(8)]
        for t in first:
            nc.vector.memset(t[:, 40, :], 1.0)
        del first  # pool rotation reuses these slots

        # ---- software-pipelined slice loop (3 stages deep)
        # conv runs TWO slices ahead of attention so the transpose DMAs of
        # slice i complete long before scores(i) need them; x loads run one
        # further slice ahead so conv never head-of-line blocks the PE queue.
        xrefs = {j: [load_x(j)] for j in range(min(3, SLAB))}
        slots = {0: [None, None, None]}
        for ch in conv_emit(xrefs[0], slots[0]):
            ch()
        if SLAB > 1:
            slots[1] = [None, None, None]
            for ch in conv_emit(xrefs[1], slots[1]):
                ch()
        pending_pbwp = []
        for i in range(SLAB):
            o_all, attn_ch = attn_emit(slots.pop(i))
            other = pending_pbwp
            if i + 2 < SLAB:
                conv_chunks = []
                if i + 3 < SLAB:
                    xrefs[i + 3] = xr = [None]
                    def ch_load(i=i, xr=xr):
                        xr[0] = load_x(i + 3)
                    conv_chunks.append(ch_load)
                slots[i + 2] = [None, None, None]
                conv_chunks += conv_emit(xrefs.pop(i + 2), slots[i + 2])
                other = _merge(pending_pbwp, conv_chunks)
            for ch in _merge(attn_ch, other):
                ch()
            pending_pbwp = pbwp_emit(o_all, i)
        for ch in pending_pbwp:
            ch()

    nc.compile()
    return nc


_NC_CACHE = None


def _get_nc():
    global _NC_CACHE
    if _NC_CACHE is None:
        _NC_CACHE = _build_nc()
    return _NC_CACHE


def make_in_maps(x, wq, bq, wk, bk, wv, bv, wp, bp):
    bf = ml_dtypes.bfloat16
    wqkv = np.concatenate(
        [wq.T * SCALE, wk.T, wv.T], axis=1
    ).astype(bf)  # [C, 3C], lhsT layout (c_in rows, c_out cols)
    bqkv = np.concatenate([bq * SCALE, bk, bv]).reshape(3 * C, 1).astype(np.float32)
    wp3 = (3.0 * wp).T.astype(bf)  # [C, C]
    bp_ = bp.reshape(C, 1).astype(np.float32)
    in_maps = []
    for core in range(N_CORES):
        b = core // 4
        r0 = (core % 4) * SLAB
        x_slab = np.ascontiguousarray(
            x[b, :, r0 : r0 + SLAB].reshape(C, NSLAB)
        ).astype(bf)
        in_maps.append(
            {"x": x_slab, "wqkv": wqkv, "bqkv": bqkv, "wp3": wp3, "bp": bp_}
        )
    return in_maps


def run_on_cores(in_maps, **kw):
    nc = _get_nc()
    return run_bass_kernel_spmd(nc, in_maps, core_ids=list(range(N_CORES)), **kw)


def kernel(x, wq, bq, wk, bk, wv, bv, wp, bp):
    x = np.asarray(x, dtype=np.float32)
    in_maps = make_in_maps(
        x,
        np.asarray(wq, np.float32),
        np.asarray(bq, np.float32),
        np.asarray(wk, np.float32),
        np.asarray(bk, np.float32),
        np.asarray(wv, np.float32),
        np.asarray(bv, np.float32),
        np.asarray(wp, np.float32),
        np.asarray(bp, np.float32),
    )
    res = run_on_cores(in_maps)
    out = np.empty((B, C, H, W, D), np.float32)
    for core in range(N_CORES):
        b = core // 4
        r0 = (core % 4) * SLAB
        out[b, :, r0 : r0 + SLAB] = res.results[core]["out"].reshape(C, SLAB, W, D)
    return out


if __name__ == "__main__":
    rng = np.random.default_rng(0)
    ins = {
        "x": rng.standard_normal((B, C, H, W, D), np.float32),
        "wq": rng.standard_normal((C, C), np.float32) / 16,
        "bq": rng.standard_normal(C).astype(np.float32) * 0.01,
        "wk": rng.standard_normal((C, C), np.float32) / 16,
        "bk": rng.standard_normal(C).astype(np.float32) * 0.01,
        "wv": rng.standard_normal((C, C), np.float32) / 16,
        "bv": rng.standard_normal(C).astype(np.float32) * 0.01,
        "wp": rng.standard_normal((C, C), np.float32) / 16,
        "bp": rng.standard_normal(C).astype(np.float32) * 0.01,
    }
    out = kernel(**ins)
    print("kernel ran, out shape", out.shape, "mean", float(np.abs(out).mean()))
